# revision 1
# baseline (speedup 1.0000x reference)
"""GridPoolingLayer kernel for Trainium2 (8 NeuronCores, Bass/Tile).

Semantics (from the grid-pooling reference): the 1D binary masks partition
H/W into maximal runs of constant value; the layer replaces every grid cell
with its mean (keep_size=True).  The op is separable: out = R @ X @ C per
channel, with R/C block "segment mean broadcast" matrices derived from the
tiny masks, which we compute on the host.

Device strategy per core (channels sharded 8 ways, 32 ch/core):
  A) row pooling   pooled1 = P_r @ X       -- PE matmul (contraction over H
     on partitions), P_r^T one-hot/len matrix precomputed host-side.
  B) col pooling   poolB = segment-sum_w   -- DVE tensor_reduce along the
     free axis.  W is pre-permuted host-side (within each super-block) so
     col segments of equal length are adjacent -> one reduce instruction
     per length class.
  C) col expand    colsDone[:, w] = poolB[:, seg(w)] / len -- DVE
     tensor_scalar_mul with a step-0 broadcast input AP, written back at
     *original* w positions (undoes the permutation on-chip).
  D) row expand    out rows = broadcast of pooled rows -- DMA straight from
     SBUF with a step-0 source AP, one DMA per row-segment (runs of
     length-1 segments merged into single multi-partition DMAs).

W is processed in NSUPER independent "super-blocks" so the resident
col-pooled tensor fits SBUF even when the row-segment count needs 3
partition chunks.  No collectives: every core runs the same program on its
channel slice.
"""

import math
import numpy as np

H, W, C = 512, 512, 256
NCORES = 8
CS = C // NCORES  # 32 channels per core
P = 128

# Tunables (w units; one w unit = CS f32 = 128B per partition)
NSUPER = 4       # independent W super-blocks
TARGET_AB = 48   # A/B-phase block width target
TARGET_CB = 64   # C/D-phase block width target
XIN_BUFS = 8
P1_BUFS = 4
CD_BUFS = 4
PB_BUFS = 2


def _segments(mask):
    m = np.asarray(mask).ravel()
    change = np.nonzero(m[1:] != m[:-1])[0] + 1
    bounds = np.concatenate([[0], change, [len(m)]]).astype(np.int64)
    return [(int(bounds[i]), int(bounds[i + 1])) for i in range(len(bounds) - 1)]


def _plan(row_segs, col_segs):
    """Host-side geometry planning shared by program build + data prep."""
    from collections import defaultdict

    S_h, S_w = len(row_segs), len(col_segs)
    Mh = math.ceil(S_h / P)
    Kh = math.ceil(H / P)

    # ---- split col segs into NSUPER contiguous groups of ~W/NSUPER w's
    supers = []
    target = W / NSUPER
    cur = []
    acc = 0
    for t, (u, v) in enumerate(col_segs):
        cur.append(t)
        acc += v - u
        if acc >= target * (len(supers) + 1) - 1e-9 and len(supers) < NSUPER - 1:
            supers.append(cur)
            cur = []
    supers.append(cur)
    supers = [s for s in supers if s]

    wperm = np.empty(W, dtype=np.int64)
    sb_plans = []
    for ts_all in supers:
        sw0 = col_segs[ts_all[0]][0]          # super start (original w)
        swid = col_segs[ts_all[-1]][1] - sw0  # super width

        by_len = defaultdict(list)
        for t in ts_all:
            u, v = col_segs[t]
            by_len[v - u].append(t)
        perm_t = [t for L in sorted(by_len) for t in by_len[L]]
        # slot[t]: column block index of seg t in this super's poolB
        slot = {t: j for j, t in enumerate(perm_t)}
        off = sw0
        for t in perm_t:
            u, v = col_segs[t]
            wperm[off:off + (v - u)] = np.arange(u, v)
            off += v - u

        # A-blocks over PERMUTED w (local to super), with class runs
        ablocks = []
        cur_b = {"w0": sw0, "wb": 0, "runs": []}
        for L in sorted(by_len):
            ts = by_len[L]
            i = 0
            while i < len(ts):
                room = max(1, (TARGET_AB - cur_b["wb"]) // L)
                take = min(room, len(ts) - i)
                cur_b["runs"].append((L, take, cur_b["wb"], slot[ts[i]]))
                cur_b["wb"] += take * L
                i += take
                if cur_b["wb"] >= TARGET_AB:
                    ablocks.append(cur_b)
                    cur_b = {"w0": cur_b["w0"] + cur_b["wb"], "wb": 0,
                             "runs": []}
        if cur_b["wb"]:
            ablocks.append(cur_b)

        # C-blocks over ORIGINAL w (local to super)
        cblocks = []
        cur_c = {"w0": sw0, "wb": 0, "ts": []}
        for t in ts_all:
            u, v = col_segs[t]
            cur_c["ts"].append(t)
            cur_c["wb"] += v - u
            if cur_c["wb"] >= TARGET_CB:
                cblocks.append(cur_c)
                cur_c = {"w0": v, "wb": 0, "ts": []}
        if cur_c["wb"]:
            cblocks.append(cur_c)

        sb_plans.append(dict(
            n_segs=len(ts_all), slot=slot,
            ablocks=ablocks, cblocks=cblocks,
        ))

    # ---- row chunk overlap: which h-chunks feed each s-chunk
    overlap = []
    for m in range(Mh):
        s_lo = m * P
        s_hi = min(S_h, (m + 1) * P)
        h_lo = row_segs[s_lo][0]
        h_hi = row_segs[s_hi - 1][1]
        ks = [k for k in range(Kh) if k * P < h_hi and (k + 1) * P > h_lo]
        overlap.append(ks)

    # ---- row expand plan: merge runs of length-1 segments
    dplan = []
    s = 0
    while s < S_h:
        a, b = row_segs[s]
        if b - a == 1:
            m, j0 = s // P, s % P
            n = 0
            while (
                s + n < S_h
                and row_segs[s + n][1] - row_segs[s + n][0] == 1
                and (s + n) // P == m
            ):
                n += 1
            dplan.append(("run1", m, j0, n, a))
            s += n
        else:
            dplan.append(("bcast", s // P, s % P, a, b - a))
            s += 1

    return dict(
        S_h=S_h, S_w=S_w, Mh=Mh, Kh=Kh,
        supers=sb_plans, overlap=overlap, dplan=dplan, wperm=wperm,
    )


def _build_program(row_segs, col_segs, plan):
    import concourse.bass as bass
    import concourse.mybir as mybir
    import concourse.tile as tile

    fp32 = mybir.dt.float32
    COPY = mybir.ActivationFunctionType.Copy
    ADD = mybir.AluOpType.add
    AXX = mybir.AxisListType.X

    Mh, Kh = plan["Mh"], plan["Kh"]
    FW = W * CS  # full row free size (16384)

    from concourse import bacc

    nc = bacc.Bacc()
    x = nc.dram_tensor("x", [H, FW], fp32, kind="ExternalInput")
    prT = nc.dram_tensor("prT", [H, Mh * P], fp32, kind="ExternalInput")
    y = nc.dram_tensor("y", [H, FW], fp32, kind="ExternalOutput")

    with tile.TileContext(nc) as tc:
        with (
            tc.tile_pool(name="consts", bufs=1) as consts,
            tc.tile_pool(name="xin", bufs=XIN_BUFS) as xin,
            tc.tile_pool(name="p1", bufs=P1_BUFS) as p1pool,
            tc.tile_pool(name="pB", bufs=PB_BUFS) as pBpool,
            tc.tile_pool(name="cd", bufs=CD_BUFS) as cdpool,
            tc.tile_pool(name="ps", bufs=6, space="PSUM") as pspool,
            tc.tile_pool(name="warm", bufs=1, space="PSUM") as warmpool,
        ):
            # stationary pooling matrices, one [P, Mh*P] tile per h-chunk
            prT_sb = []
            for k in range(Kh):
                t = consts.tile([P, Mh * P], fp32, name=f"prT{k}")
                nc.sync.dma_start(t[:], prT[k * P:(k + 1) * P, :])
                prT_sb.append(t)

            # PE pre-touch of every prT tile: later matmuls then reach the
            # stationary operand without a DMA wait (keeps the LDWEIGHTS
            # sync-wait count within the ISA limit).
            ps_warm = warmpool.tile([1, 512], fp32, name="ps_warm")
            for k in range(Kh):
                nc.tensor.matmul(
                    ps_warm[:1, :1],
                    prT_sb[k][:, :1],
                    prT_sb[k][:, :1],
                    start=True,
                    stop=True,
                )

            for si, sp in enumerate(plan["supers"]):
                # this super's col-pooled tensor, one tile per s-chunk
                poolB = [
                    pBpool.tile([P, sp["n_segs"] * CS], fp32, tag=f"pB{m}",
                                name=f"poolB{si}_{m}")
                    for m in range(Mh)
                ]

                # ---------------- phase A+B ----------------
                for bi, blk in enumerate(sp["ablocks"]):
                    wb = blk["wb"]
                    fw = wb * CS
                    xts = []
                    for k in range(Kh):
                        xt = xin.tile([P, fw], fp32, tag="xt",
                                      name=f"xt{si}_{bi}_{k}")
                        nc.sync.dma_start(
                            xt[:],
                            x[k * P:(k + 1) * P,
                              blk["w0"] * CS:(blk["w0"] + wb) * CS],
                        )
                        nc.tensor.matmul(
                            ps_warm[:1, :1],
                            xt[:, :1],
                            xt[:, :1],
                            start=True,
                            stop=True,
                        )
                        xts.append(xt)
                    for m in range(Mh):
                        p1 = p1pool.tile([P, fw], fp32, tag="p1",
                                         name=f"p1_{si}_{bi}_{m}")
                        ks = plan["overlap"][m]
                        for n0 in range(0, fw, 512):
                            nw = min(512, fw - n0)
                            ps = pspool.tile([P, 512], fp32, tag="ps",
                                             name=f"ps{si}_{bi}_{m}_{n0}")
                            for i, k in enumerate(ks):
                                nc.tensor.matmul(
                                    ps[:, :nw],
                                    prT_sb[k][:, m * P:(m + 1) * P],
                                    xts[k][:, n0:n0 + nw],
                                    start=(i == 0),
                                    stop=(i == len(ks) - 1),
                                )
                            nc.scalar.activation(p1[:, n0:n0 + nw],
                                                 ps[:, :nw], COPY)
                        # stage B: one reduce per class-run
                        for (L, n, lw0, slot0) in blk["runs"]:
                            src = p1[:, lw0 * CS:(lw0 + n * L) * CS]
                            src = src.rearrange(
                                "p (j l c) -> p j c l", j=n, l=L, c=CS
                            )
                            dst = poolB[m][:, slot0 * CS:(slot0 + n) * CS]
                            dst = dst.rearrange("p (j c) -> p j c", j=n, c=CS)
                            nc.vector.tensor_reduce(dst, src, axis=AXX, op=ADD)

                # ---------------- phase C+D ----------------
                for ci, cblk in enumerate(sp["cblocks"]):
                    cw = cblk["wb"]
                    fcw = cw * CS
                    for m in range(Mh):
                        cd = cdpool.tile([P, fcw], fp32, tag="cd",
                                         name=f"cd{si}_{ci}_{m}")
                        for t in cblk["ts"]:
                            u, v = col_segs[t]
                            L = v - u
                            lw0 = u - cblk["w0"]
                            sl = sp["slot"][t]
                            src = poolB[m][:, sl * CS:(sl + 1) * CS]
                            dst = cd[:, lw0 * CS:(lw0 + L) * CS]
                            if L == 1:
                                nc.vector.tensor_scalar_mul(dst, src, 1.0)
                            else:
                                srcb = src.unsqueeze(1).broadcast_to(
                                    [P, L, CS])
                                dstr = dst.rearrange("p (l c) -> p l c",
                                                     l=L, c=CS)
                                nc.vector.tensor_scalar_mul(dstr, srcb,
                                                            1.0 / L)
                        # stage D for this (cblock, m)
                        c0 = cblk["w0"] * CS
                        for entry in plan["dplan"]:
                            if entry[0] == "run1":
                                _, em, j0, n, h0 = entry
                                if em != m:
                                    continue
                                nc.sync.dma_start(
                                    y[h0:h0 + n, c0:c0 + fcw],
                                    cd[j0:j0 + n, :],
                                )
                            else:
                                _, em, j, h0, L = entry
                                if em != m:
                                    continue
                                src = cd[j:j + 1, :].unsqueeze(1)
                                src = src.broadcast_to([1, L, fcw])
                                nc.sync.dma_start(
                                    y[h0:h0 + L, c0:c0 + fcw], src
                                )

    nc.compile()
    nc.finalize()
    return nc


def _prep_host(input, h_mask, v_mask):
    """Returns (nc, in_maps, plan) ready for execution."""
    row_segs = _segments(h_mask)
    col_segs = _segments(v_mask)
    plan = _plan(row_segs, col_segs)

    # pooling matrix P_r^T with 1/count folded in
    Mh = plan["Mh"]
    prT = np.zeros((H, Mh * P), dtype=np.float32)
    for s, (a, b) in enumerate(row_segs):
        prT[a:b, s] = 1.0 / (b - a)

    # host W permutation (class-sorted within supers), per-core channel slices
    xp = np.ascontiguousarray(input[0][:, plan["wperm"], :])  # [H, W, C]
    in_maps = []
    for k in range(NCORES):
        xc = np.ascontiguousarray(xp[:, :, k * CS:(k + 1) * CS])
        in_maps.append({"x": xc.reshape(H, W * CS), "prT": prT})

    nc = _build_program(row_segs, col_segs, plan)
    return nc, in_maps, plan


# stash for test.py introspection
LAST_RESULT = {}
_EXEC_CACHE = {}


def _make_executable(nc):
    """Build a reusable sharded jit callable for this program.

    Mirrors bass2jax.run_bass_via_pjrt's multi-core branch but keeps the
    jitted function so repeated calls skip retrace/recompile (and so the
    test harness can time steady-state executions).
    """
    import jax
    import concourse.mybir as mybir
    from concourse import bass2jax
    from jax.sharding import Mesh, PartitionSpec
    from jax.experimental.shard_map import shard_map

    bass2jax.install_neuronx_cc_hook()

    partition_name = (
        nc.partition_id_tensor.name if nc.partition_id_tensor else None
    )
    in_names, out_names, out_shapes, out_dtypes = [], [], [], []
    for alloc in nc.m.functions[0].allocations:
        if not isinstance(alloc, mybir.MemoryLocationSet):
            continue
        name = alloc.memorylocations[0].name
        if alloc.kind == "ExternalInput":
            if name != partition_name:
                in_names.append(name)
        elif alloc.kind == "ExternalOutput":
            out_names.append(name)
            out_shapes.append(tuple(alloc.tensor_shape))
            out_dtypes.append(mybir.dt.np(alloc.dtype))
    out_avals = tuple(
        jax.core.ShapedArray(s, d) for s, d in zip(out_shapes, out_dtypes)
    )
    n_params = len(in_names)
    n_outs = len(out_names)
    all_names = in_names + out_names
    if partition_name is not None:
        all_names = all_names + [partition_name]

    def _body(*args):
        operands = list(args)
        if partition_name is not None:
            operands.append(bass2jax.partition_id_tensor())
        outs = bass2jax._bass_exec_p.bind(
            *operands,
            out_avals=out_avals,
            in_names=tuple(all_names),
            out_names=tuple(out_names),
            lowering_input_output_aliases=(),
            sim_require_finite=True,
            sim_require_nnan=True,
            nc=nc,
        )
        return tuple(outs)

    devices = jax.devices()[:NCORES]
    mesh = Mesh(np.asarray(devices), ("core",))
    donate = tuple(range(n_params, n_params + n_outs))
    sharded = jax.jit(
        shard_map(
            _body,
            mesh=mesh,
            in_specs=(PartitionSpec("core"),) * (n_params + n_outs),
            out_specs=(PartitionSpec("core"),) * n_outs,
            check_rep=False,
        ),
        donate_argnums=donate,
        keep_unused=True,
    )

    def run(in_maps):
        concat_in = [
            np.concatenate([m[name] for m in in_maps], axis=0)
            for name in in_names
        ]
        concat_zeros = [
            np.zeros((NCORES * s[0], *s[1:]), d)
            for s, d in zip(out_shapes, out_dtypes)
        ]
        out_arrs = sharded(*concat_in, *concat_zeros)
        return [
            {
                name: np.asarray(out_arrs[i]).reshape(
                    NCORES, *out_shapes[i]
                )[c]
                for i, name in enumerate(out_names)
            }
            for c in range(NCORES)
        ]

    return run


def _get_run(input, h_mask, v_mask):
    key = (np.asarray(h_mask).tobytes(), np.asarray(v_mask).tobytes())
    if key not in _EXEC_CACHE:
        nc, in_maps, plan = _prep_host(
            np.asarray(input), np.asarray(h_mask), np.asarray(v_mask)
        )
        _EXEC_CACHE[key] = (_make_executable(nc), plan)
    else:
        # still need per-call input prep (data may differ between calls)
        row_segs = _segments(h_mask)
        col_segs = _segments(v_mask)
        plan = _EXEC_CACHE[key][1]
        Mh = plan["Mh"]
        prT = np.zeros((H, Mh * P), dtype=np.float32)
        for s, (a, b) in enumerate(row_segs):
            prT[a:b, s] = 1.0 / (b - a)
        xp = np.ascontiguousarray(np.asarray(input)[0][:, plan["wperm"], :])
        in_maps = [
            {
                "x": np.ascontiguousarray(
                    xp[:, :, k * CS:(k + 1) * CS]
                ).reshape(H, W * CS),
                "prT": prT,
            }
            for k in range(NCORES)
        ]
    return _EXEC_CACHE[key][0], in_maps


def kernel(input, h_mask, v_mask):
    run, in_maps = _get_run(input, h_mask, v_mask)
    results = run(in_maps)
    LAST_RESULT["results"] = results
    out = np.concatenate(
        [results[k]["y"].reshape(H, W, CS) for k in range(NCORES)],
        axis=-1,
    )
    return out[None].astype(np.float32)



# revision 2
# speedup vs baseline: 9560.4278x; 9560.4278x over previous
"""GridPoolingLayer kernel for Trainium2 (8 NeuronCores, Bass/Tile).

Semantics (from the grid-pooling reference): the 1D binary masks partition
H/W into maximal runs of constant value; the layer replaces every grid cell
with its mean (keep_size=True).  The op is separable: out = R @ X @ C per
channel, with R/C block "segment mean broadcast" matrices derived from the
tiny masks, which we compute on the host.

Device strategy per core (channels sharded 8 ways, 32 ch/core):
  A) row pooling   pooled1 = P_r @ X       -- PE matmul (contraction over H
     on partitions), P_r^T one-hot/len matrix precomputed host-side.
  B) col pooling   poolB = segment-sum_w   -- DVE tensor_reduce along the
     free axis.  W is pre-permuted host-side (within each super-block) so
     col segments of equal length are adjacent -> one reduce instruction
     per length class.
  C) col expand    colsDone[:, w] = poolB[:, seg(w)] / len -- DVE
     tensor_scalar_mul with a step-0 broadcast input AP, written back at
     *original* w positions (undoes the permutation on-chip).
  D) row expand    out rows = broadcast of pooled rows -- DMA straight from
     SBUF with a step-0 source AP, one DMA per row-segment (runs of
     length-1 segments merged into single multi-partition DMAs).

W is processed in NSUPER independent "super-blocks" so the resident
col-pooled tensor fits SBUF even when the row-segment count needs 3
partition chunks.  No collectives: every core runs the same program on its
channel slice.
"""

import math
import numpy as np

H, W, C = 512, 512, 256
NCORES = 8
CS = C // NCORES  # 32 channels per core
P = 128

# Tunables (w units; one w unit = CS f32 = 128B per partition)
NSUPER = 4       # independent W super-blocks
TARGET_AB = 48   # A/B-phase block width target
TARGET_CB = 64   # C/D-phase block width target
XIN_BUFS = 8
P1_BUFS = 4
CD_BUFS = 4
PB_BUFS = 2


def _segments(mask):
    m = np.asarray(mask).ravel()
    change = np.nonzero(m[1:] != m[:-1])[0] + 1
    bounds = np.concatenate([[0], change, [len(m)]]).astype(np.int64)
    return [(int(bounds[i]), int(bounds[i + 1])) for i in range(len(bounds) - 1)]


def _plan(row_segs, col_segs):
    """Host-side geometry planning shared by program build + data prep."""
    from collections import defaultdict

    S_h, S_w = len(row_segs), len(col_segs)
    Mh = math.ceil(S_h / P)
    Kh = math.ceil(H / P)

    # ---- split col segs into NSUPER contiguous groups of ~W/NSUPER w's
    supers = []
    target = W / NSUPER
    cur = []
    acc = 0
    for t, (u, v) in enumerate(col_segs):
        cur.append(t)
        acc += v - u
        if acc >= target * (len(supers) + 1) - 1e-9 and len(supers) < NSUPER - 1:
            supers.append(cur)
            cur = []
    supers.append(cur)
    supers = [s for s in supers if s]

    wperm = np.empty(W, dtype=np.int64)
    sb_plans = []
    for ts_all in supers:
        sw0 = col_segs[ts_all[0]][0]          # super start (original w)
        swid = col_segs[ts_all[-1]][1] - sw0  # super width

        by_len = defaultdict(list)
        for t in ts_all:
            u, v = col_segs[t]
            by_len[v - u].append(t)
        perm_t = [t for L in sorted(by_len) for t in by_len[L]]
        # slot[t]: column block index of seg t in this super's poolB
        slot = {t: j for j, t in enumerate(perm_t)}
        off = sw0
        for t in perm_t:
            u, v = col_segs[t]
            wperm[off:off + (v - u)] = np.arange(u, v)
            off += v - u

        # A-blocks over PERMUTED w (local to super), with class runs
        ablocks = []
        cur_b = {"w0": sw0, "wb": 0, "runs": []}
        for L in sorted(by_len):
            ts = by_len[L]
            i = 0
            while i < len(ts):
                room = max(1, (TARGET_AB - cur_b["wb"]) // L)
                take = min(room, len(ts) - i)
                cur_b["runs"].append((L, take, cur_b["wb"], slot[ts[i]]))
                cur_b["wb"] += take * L
                i += take
                if cur_b["wb"] >= TARGET_AB:
                    ablocks.append(cur_b)
                    cur_b = {"w0": cur_b["w0"] + cur_b["wb"], "wb": 0,
                             "runs": []}
        if cur_b["wb"]:
            ablocks.append(cur_b)

        # C-blocks over ORIGINAL w (local to super)
        cblocks = []
        cur_c = {"w0": sw0, "wb": 0, "ts": []}
        for t in ts_all:
            u, v = col_segs[t]
            cur_c["ts"].append(t)
            cur_c["wb"] += v - u
            if cur_c["wb"] >= TARGET_CB:
                cblocks.append(cur_c)
                cur_c = {"w0": v, "wb": 0, "ts": []}
        if cur_c["wb"]:
            cblocks.append(cur_c)

        sb_plans.append(dict(
            n_segs=len(ts_all), slot=slot,
            ablocks=ablocks, cblocks=cblocks,
        ))

    # ---- row chunk overlap: which h-chunks feed each s-chunk
    overlap = []
    for m in range(Mh):
        s_lo = m * P
        s_hi = min(S_h, (m + 1) * P)
        h_lo = row_segs[s_lo][0]
        h_hi = row_segs[s_hi - 1][1]
        ks = [k for k in range(Kh) if k * P < h_hi and (k + 1) * P > h_lo]
        overlap.append(ks)

    # ---- row expand plan: merge runs of length-1 segments
    dplan = []
    s = 0
    while s < S_h:
        a, b = row_segs[s]
        if b - a == 1:
            m, j0 = s // P, s % P
            n = 0
            while (
                s + n < S_h
                and row_segs[s + n][1] - row_segs[s + n][0] == 1
                and (s + n) // P == m
            ):
                n += 1
            dplan.append(("run1", m, j0, n, a))
            s += n
        else:
            dplan.append(("bcast", s // P, s % P, a, b - a))
            s += 1

    return dict(
        S_h=S_h, S_w=S_w, Mh=Mh, Kh=Kh,
        supers=sb_plans, overlap=overlap, dplan=dplan, wperm=wperm,
    )


def _build_program(row_segs, col_segs, plan):
    import concourse.bass as bass
    import concourse.mybir as mybir
    import concourse.tile as tile

    fp32 = mybir.dt.float32
    COPY = mybir.ActivationFunctionType.Copy
    ADD = mybir.AluOpType.add
    AXX = mybir.AxisListType.X

    Mh, Kh = plan["Mh"], plan["Kh"]
    FW = W * CS  # full row free size (16384)

    from concourse import bacc

    nc = bacc.Bacc()
    x = nc.dram_tensor("x", [H, FW], fp32, kind="ExternalInput")
    prT = nc.dram_tensor("prT", [H, Mh * P], fp32, kind="ExternalInput")
    y = nc.dram_tensor("y", [H, FW], fp32, kind="ExternalOutput")

    with tile.TileContext(nc) as tc:
        with (
            tc.tile_pool(name="consts", bufs=1) as consts,
            tc.tile_pool(name="xin", bufs=XIN_BUFS) as xin,
            tc.tile_pool(name="p1", bufs=P1_BUFS) as p1pool,
            tc.tile_pool(name="pB", bufs=PB_BUFS) as pBpool,
            tc.tile_pool(name="cd", bufs=CD_BUFS) as cdpool,
            tc.tile_pool(name="ps", bufs=6, space="PSUM") as pspool,
            tc.tile_pool(name="warm", bufs=1, space="PSUM") as warmpool,
        ):
            # stationary pooling matrices, one [P, Mh*P] tile per h-chunk
            prT_sb = []
            for k in range(Kh):
                t = consts.tile([P, Mh * P], fp32, name=f"prT{k}")
                nc.sync.dma_start(t[:], prT[k * P:(k + 1) * P, :])
                prT_sb.append(t)

            # PE pre-touch of every prT tile: later matmuls then reach the
            # stationary operand without a DMA wait (keeps the LDWEIGHTS
            # sync-wait count within the ISA limit).
            ps_warm = warmpool.tile([1, 512], fp32, name="ps_warm")
            for k in range(Kh):
                nc.tensor.matmul(
                    ps_warm[:1, :1],
                    prT_sb[k][:, :1],
                    prT_sb[k][:, :1],
                    start=True,
                    stop=True,
                )

            for si, sp in enumerate(plan["supers"]):
                # this super's col-pooled tensor, one tile per s-chunk
                poolB = [
                    pBpool.tile([P, sp["n_segs"] * CS], fp32, tag=f"pB{m}",
                                name=f"poolB{si}_{m}")
                    for m in range(Mh)
                ]

                # ---------------- phase A+B ----------------
                for bi, blk in enumerate(sp["ablocks"]):
                    wb = blk["wb"]
                    fw = wb * CS
                    xts = []
                    for k in range(Kh):
                        xt = xin.tile([P, fw], fp32, tag="xt",
                                      name=f"xt{si}_{bi}_{k}")
                        nc.sync.dma_start(
                            xt[:],
                            x[k * P:(k + 1) * P,
                              blk["w0"] * CS:(blk["w0"] + wb) * CS],
                        )
                        nc.tensor.matmul(
                            ps_warm[:1, :1],
                            xt[:, :1],
                            xt[:, :1],
                            start=True,
                            stop=True,
                        )
                        xts.append(xt)
                    for m in range(Mh):
                        p1 = p1pool.tile([P, fw], fp32, tag="p1",
                                         name=f"p1_{si}_{bi}_{m}")
                        ks = plan["overlap"][m]
                        for n0 in range(0, fw, 512):
                            nw = min(512, fw - n0)
                            ps = pspool.tile([P, 512], fp32, tag="ps",
                                             name=f"ps{si}_{bi}_{m}_{n0}")
                            for i, k in enumerate(ks):
                                nc.tensor.matmul(
                                    ps[:, :nw],
                                    prT_sb[k][:, m * P:(m + 1) * P],
                                    xts[k][:, n0:n0 + nw],
                                    start=(i == 0),
                                    stop=(i == len(ks) - 1),
                                )
                            nc.scalar.activation(p1[:, n0:n0 + nw],
                                                 ps[:, :nw], COPY)
                        # stage B: one reduce per class-run
                        for (L, n, lw0, slot0) in blk["runs"]:
                            src = p1[:, lw0 * CS:(lw0 + n * L) * CS]
                            src = src.rearrange(
                                "p (j l c) -> p j c l", j=n, l=L, c=CS
                            )
                            dst = poolB[m][:, slot0 * CS:(slot0 + n) * CS]
                            dst = dst.rearrange("p (j c) -> p j c", j=n, c=CS)
                            nc.vector.tensor_reduce(dst, src, axis=AXX, op=ADD)

                # ---------------- phase C+D ----------------
                for ci, cblk in enumerate(sp["cblocks"]):
                    cw = cblk["wb"]
                    fcw = cw * CS
                    for m in range(Mh):
                        cd = cdpool.tile([P, fcw], fp32, tag="cd",
                                         name=f"cd{si}_{ci}_{m}")
                        for t in cblk["ts"]:
                            u, v = col_segs[t]
                            L = v - u
                            lw0 = u - cblk["w0"]
                            sl = sp["slot"][t]
                            src = poolB[m][:, sl * CS:(sl + 1) * CS]
                            dst = cd[:, lw0 * CS:(lw0 + L) * CS]
                            if L == 1:
                                nc.vector.tensor_scalar_mul(dst, src, 1.0)
                            else:
                                srcb = src.unsqueeze(1).broadcast_to(
                                    [P, L, CS])
                                dstr = dst.rearrange("p (l c) -> p l c",
                                                     l=L, c=CS)
                                nc.vector.tensor_scalar_mul(dstr, srcb,
                                                            1.0 / L)
                        # stage D for this (cblock, m)
                        c0 = cblk["w0"] * CS
                        for entry in plan["dplan"]:
                            if entry[0] == "run1":
                                _, em, j0, n, h0 = entry
                                if em != m:
                                    continue
                                nc.sync.dma_start(
                                    y[h0:h0 + n, c0:c0 + fcw],
                                    cd[j0:j0 + n, :],
                                )
                            else:
                                _, em, j, h0, L = entry
                                if em != m:
                                    continue
                                src = cd[j:j + 1, :].unsqueeze(1)
                                src = src.broadcast_to([1, L, fcw])
                                nc.sync.dma_start(
                                    y[h0:h0 + L, c0:c0 + fcw], src
                                )

    nc.compile()
    nc.finalize()
    return nc


def _prep_host(input, h_mask, v_mask):
    """Returns (nc, in_maps, plan) ready for execution."""
    row_segs = _segments(h_mask)
    col_segs = _segments(v_mask)
    plan = _plan(row_segs, col_segs)

    # pooling matrix P_r^T with 1/count folded in
    Mh = plan["Mh"]
    prT = np.zeros((H, Mh * P), dtype=np.float32)
    for s, (a, b) in enumerate(row_segs):
        prT[a:b, s] = 1.0 / (b - a)

    # host W permutation (class-sorted within supers), per-core channel slices
    xp = np.ascontiguousarray(input[0][:, plan["wperm"], :])  # [H, W, C]
    in_maps = []
    for k in range(NCORES):
        xc = np.ascontiguousarray(xp[:, :, k * CS:(k + 1) * CS])
        in_maps.append({"x": xc.reshape(H, W * CS), "prT": prT})

    nc = _build_program(row_segs, col_segs, plan)
    return nc, in_maps, plan


# stash for test.py introspection
LAST_RESULT = {}
_EXEC_CACHE = {}


def _make_executable(nc):
    """Build a reusable sharded jit callable for this program.

    Mirrors bass2jax.run_bass_via_pjrt's multi-core branch but keeps the
    jitted function so repeated calls skip retrace/recompile (and so the
    test harness can time steady-state executions).
    """
    import jax
    import concourse.mybir as mybir
    from concourse import bass2jax
    from jax.sharding import Mesh, PartitionSpec
    from jax.experimental.shard_map import shard_map

    bass2jax.install_neuronx_cc_hook()

    partition_name = (
        nc.partition_id_tensor.name if nc.partition_id_tensor else None
    )
    in_names, out_names, out_shapes, out_dtypes = [], [], [], []
    for alloc in nc.m.functions[0].allocations:
        if not isinstance(alloc, mybir.MemoryLocationSet):
            continue
        name = alloc.memorylocations[0].name
        if alloc.kind == "ExternalInput":
            if name != partition_name:
                in_names.append(name)
        elif alloc.kind == "ExternalOutput":
            out_names.append(name)
            out_shapes.append(tuple(alloc.tensor_shape))
            out_dtypes.append(mybir.dt.np(alloc.dtype))
    out_avals = tuple(
        jax.core.ShapedArray(s, d) for s, d in zip(out_shapes, out_dtypes)
    )
    n_params = len(in_names)
    n_outs = len(out_names)
    all_names = in_names + out_names
    if partition_name is not None:
        all_names = all_names + [partition_name]

    def _body(*args):
        operands = list(args)
        if partition_name is not None:
            operands.append(bass2jax.partition_id_tensor())
        outs = bass2jax._bass_exec_p.bind(
            *operands,
            out_avals=out_avals,
            in_names=tuple(all_names),
            out_names=tuple(out_names),
            lowering_input_output_aliases=(),
            sim_require_finite=True,
            sim_require_nnan=True,
            nc=nc,
        )
        return tuple(outs)

    devices = jax.devices()[:NCORES]
    mesh = Mesh(np.asarray(devices), ("core",))
    donate = tuple(range(n_params, n_params + n_outs))
    sharded = jax.jit(
        shard_map(
            _body,
            mesh=mesh,
            in_specs=(PartitionSpec("core"),) * (n_params + n_outs),
            out_specs=(PartitionSpec("core"),) * n_outs,
            check_rep=False,
        ),
        donate_argnums=donate,
        keep_unused=True,
    )

    def run(in_maps):
        concat_in = [
            np.concatenate([m[name] for m in in_maps], axis=0)
            for name in in_names
        ]
        concat_zeros = [
            np.zeros((NCORES * s[0], *s[1:]), d)
            for s, d in zip(out_shapes, out_dtypes)
        ]
        out_arrs = sharded(*concat_in, *concat_zeros)
        return [
            {
                name: np.asarray(out_arrs[i]).reshape(
                    NCORES, *out_shapes[i]
                )[c]
                for i, name in enumerate(out_names)
            }
            for c in range(NCORES)
        ]

    return run


def _get_run(input, h_mask, v_mask):
    key = (np.asarray(h_mask).tobytes(), np.asarray(v_mask).tobytes())
    if key not in _EXEC_CACHE:
        nc, in_maps, plan = _prep_host(
            np.asarray(input), np.asarray(h_mask), np.asarray(v_mask)
        )
        LAST_RESULT["nc"] = nc
        _EXEC_CACHE[key] = (_make_executable(nc), plan)
    else:
        # still need per-call input prep (data may differ between calls)
        row_segs = _segments(h_mask)
        col_segs = _segments(v_mask)
        plan = _EXEC_CACHE[key][1]
        Mh = plan["Mh"]
        prT = np.zeros((H, Mh * P), dtype=np.float32)
        for s, (a, b) in enumerate(row_segs):
            prT[a:b, s] = 1.0 / (b - a)
        xp = np.ascontiguousarray(np.asarray(input)[0][:, plan["wperm"], :])
        in_maps = [
            {
                "x": np.ascontiguousarray(
                    xp[:, :, k * CS:(k + 1) * CS]
                ).reshape(H, W * CS),
                "prT": prT,
            }
            for k in range(NCORES)
        ]
    return _EXEC_CACHE[key][0], in_maps


def kernel(input, h_mask, v_mask):
    run, in_maps = _get_run(input, h_mask, v_mask)
    results = run(in_maps)
    LAST_RESULT["results"] = results
    out = np.concatenate(
        [results[k]["y"].reshape(H, W, CS) for k in range(NCORES)],
        axis=-1,
    )
    return out[None].astype(np.float32)



# revision 4
# speedup vs baseline: 27807.3924x; 2.9086x over previous
"""GridPoolingLayer kernel for Trainium2 (8 NeuronCores, Bass/Tile).

Semantics: the 1D binary masks partition H/W into maximal runs of constant
value; the layer replaces every grid cell with its mean (keep_size=True).
The op is separable: col-segment-mean along W, then row-segment-mean along
H, then broadcast back over each cell.

Device strategy per core (channels sharded 8 ways, 32 ch/core), fp16:

  B) col pooling   cp[k] = segment-sum_w(x chunk k)   -- DVE tensor_reduce
     along the free axis.  W is pre-permuted host-side so col segments of
     equal length are adjacent -> one reduce instruction per length class
     per load block.
  A) row pooling   pooled = P_r^T @ cp                -- PE matmul
     (contraction over H on partitions), P_r one-hot/len fp16 matrix
     precomputed host-side; row segments are permuted within each
     128-segment chunk so equal lengths are adjacent.
  C) col expand    rowtile[:, w] = pooled[:, seg(w)] / len_w -- DVE
     tensor_scalar_mul reading PSUM directly with a step-0 broadcast AP,
     one instruction per (length class x PSUM tile) piece.
  D) row expand    y rows = broadcast of pooled rows  -- DMA straight
     from SBUF with a step-0 source AP, ONE DMA per (s-chunk x row length
     class): output rows are written in class-grouped order.

The host un-permutes both axes (pure gathers) while unsharding and
upcasts fp16 -> fp32.  fp16 keeps HBM traffic at 16 MB in + 16 MB out
per core (vs 64 MB for fp32) and runs the PE at full 16-bit rate; the
2e-2 harness tolerance leaves ~40x margin over fp16 rounding noise.
"""

import math
import numpy as np

H, W, C = 512, 512, 256
NCORES = 8
CS = C // NCORES  # 32 channels per core
P = 128
FW = W * CS       # full row free size in elements (16384)
PSW = 512         # psum tile width (fp32 elems, one bank)
SLOTS_PER_TILE = PSW // CS  # 16 col segments per psum tile

TB = 128          # x load block target width (w units; 128 -> 1MB DMAs)
XIN_BUFS = 3
RT_BUFS = 2
PS_BUFS = 7


def _segments(mask):
    m = np.asarray(mask).ravel()
    change = np.nonzero(m[1:] != m[:-1])[0] + 1
    bounds = np.concatenate([[0], change, [len(m)]]).astype(np.int64)
    return [(int(bounds[i]), int(bounds[i + 1])) for i in range(len(bounds) - 1)]


def _plan(row_segs, col_segs):
    """Host-side geometry planning shared by program build + data prep."""
    from collections import defaultdict

    S_h, S_w = len(row_segs), len(col_segs)
    Mh = math.ceil(S_h / P)
    Kh = math.ceil(H / P)

    # ---- column side: class-sorted device order -------------------------
    by_len = defaultdict(list)
    for t, (u, v) in enumerate(col_segs):
        by_len[v - u].append(t)
    perm_cols = [t for L in sorted(by_len) for t in by_len[L]]

    wperm = np.empty(W, dtype=np.int64)   # dev w unit -> orig w
    wstart = np.empty(S_w + 1, dtype=np.int64)  # dev slot -> dev w unit
    off = 0
    for sl, t in enumerate(perm_cols):
        u, v = col_segs[t]
        wstart[sl] = off
        wperm[off:off + (v - u)] = np.arange(u, v)
        off += v - u
    wstart[S_w] = off
    assert off == W

    # class runs in device slot order: (L, n, lw0, slot0)
    col_runs = []
    sl = 0
    for L in sorted(by_len):
        n = len(by_len[L])
        col_runs.append((L, n, int(wstart[sl]), sl))
        sl += n

    # ---- x load blocks: split class runs at ~TB w units -----------------
    load_blocks = []
    cur = {"w0": 0, "wb": 0, "runs": []}
    for (L, n, lw0, slot0) in col_runs:
        i = 0
        while i < n:
            room = max(1, (TB - cur["wb"]) // L)
            take = min(room, n - i)
            cur["runs"].append((L, take, cur["wb"], slot0 + i))
            cur["wb"] += take * L
            i += take
            if cur["wb"] >= TB:
                load_blocks.append(cur)
                cur = {"w0": cur["w0"] + cur["wb"], "wb": 0, "runs": []}
    if cur["wb"]:
        load_blocks.append(cur)

    # ---- C pieces: class runs split at psum tile boundaries -------------
    # piece: (tile_idx, L, n, slot0, lw0)
    c_pieces = defaultdict(list)
    for (L, n, lw0, slot0) in col_runs:
        i = 0
        while i < n:
            t_idx = (slot0 + i) // SLOTS_PER_TILE
            room = (t_idx + 1) * SLOTS_PER_TILE - (slot0 + i)
            take = min(room, n - i)
            c_pieces[t_idx].append(
                (L, take, slot0 + i, int(wstart[slot0 + i]))
            )
            i += take

    n_tiles = math.ceil(S_w * CS / PSW)

    # ---- row side: class-sorted order within each s-chunk ---------------
    # dev_rows[r] = orig h; d_runs[m] = [(r0, n, L, j0)]
    dev_rows = np.empty(H, dtype=np.int64)
    d_runs = [[] for _ in range(Mh)]
    seg_perm = []  # global seg order: for m, class-sorted within chunk
    r0 = 0
    for m in range(Mh):
        chunk = list(range(m * P, min(S_h, (m + 1) * P)))
        chunk.sort(key=lambda s: (row_segs[s][1] - row_segs[s][0], s))
        j = 0
        while j < len(chunk):
            L = row_segs[chunk[j]][1] - row_segs[chunk[j]][0]
            n = 0
            while j + n < len(chunk) and (
                row_segs[chunk[j + n]][1] - row_segs[chunk[j + n]][0] == L
            ):
                n += 1
            d_runs[m].append((r0, n, L, j))
            for jj in range(n):
                a, b = row_segs[chunk[j + jj]]
                dev_rows[r0:r0 + L] = np.arange(a, b)
                r0 += L
            j += n
        seg_perm.extend(chunk)
    assert r0 == H

    # ---- row chunk overlap: which h-chunks feed each s-chunk ------------
    overlap = []
    for m in range(Mh):
        s_lo, s_hi = m * P, min(S_h, (m + 1) * P)
        h_lo = min(row_segs[s][0] for s in range(s_lo, s_hi))
        h_hi = max(row_segs[s][1] for s in range(s_lo, s_hi))
        overlap.append(
            [k for k in range(Kh) if k * P < h_hi and (k + 1) * P > h_lo]
        )

    return dict(
        S_h=S_h, S_w=S_w, Mh=Mh, Kh=Kh,
        wperm=wperm, col_runs=col_runs, load_blocks=load_blocks,
        c_pieces=c_pieces, n_tiles=n_tiles,
        dev_rows=dev_rows, d_runs=d_runs, seg_perm=seg_perm,
        overlap=overlap,
    )


def _build_program(row_segs, col_segs, plan):
    import concourse.bass as bass
    import concourse.mybir as mybir
    import concourse.tile as tile
    from concourse import bacc

    fp16 = mybir.dt.float16
    fp32 = mybir.dt.float32
    ADD = mybir.AluOpType.add
    AXX = mybir.AxisListType.X

    Mh, Kh = plan["Mh"], plan["Kh"]
    S_w = plan["S_w"]
    CPW = S_w * CS  # col-pooled row free size

    nc = bacc.Bacc()
    x = nc.dram_tensor("x", [H, FW], fp16, kind="ExternalInput")
    prT = nc.dram_tensor("prT", [H, Mh * P], fp16, kind="ExternalInput")
    y = nc.dram_tensor("y", [H, FW], fp16, kind="ExternalOutput")

    with tile.TileContext(nc) as tc:
        with (
            tc.tile_pool(name="consts", bufs=1) as consts,
            tc.tile_pool(name="xin", bufs=XIN_BUFS) as xin,
            tc.tile_pool(name="cp", bufs=1) as cppool,
            tc.tile_pool(name="rt", bufs=RT_BUFS) as rtpool,
            tc.tile_pool(name="ps", bufs=PS_BUFS, space="PSUM") as pspool,
            tc.tile_pool(name="warm", bufs=1, space="PSUM") as warmpool,
        ):
            # stationary pooling matrices, one [P, Mh*P] tile per h-chunk
            prT_sb = []
            for k in range(Kh):
                t = consts.tile([P, Mh * P], fp16, name=f"prT{k}")
                nc.sync.dma_start(t[:], prT[k * P:(k + 1) * P, :])
                prT_sb.append(t)

            # PE pre-touch of every prT tile: later matmuls then reach the
            # stationary operand without a DMA wait (keeps the LDWEIGHTS
            # sync-wait count within the ISA limit).
            ps_warm = warmpool.tile([1, PSW], fp32, name="ps_warm")
            for k in range(Kh):
                nc.tensor.matmul(
                    ps_warm[:1, :1],
                    prT_sb[k][:, :1],
                    prT_sb[k][:, :1],
                    start=True,
                    stop=True,
                )

            # ------------- phase B: load + col segment-sum ---------------
            cp = [
                cppool.tile([P, CPW], fp16, name=f"cp{k}") for k in range(Kh)
            ]
            for k in range(Kh):
                for bi, blk in enumerate(plan["load_blocks"]):
                    wb = blk["wb"]
                    xt = xin.tile([P, wb * CS], fp16, tag="xt",
                                  name=f"xt{k}_{bi}")
                    nc.sync.dma_start(
                        xt[:],
                        x[k * P:(k + 1) * P,
                          blk["w0"] * CS:(blk["w0"] + wb) * CS],
                    )
                    for (L, n, lw0, slot0) in blk["runs"]:
                        src = xt[:, lw0 * CS:(lw0 + n * L) * CS]
                        src = src.rearrange(
                            "p (j l c) -> p j c l", j=n, l=L, c=CS
                        )
                        dst = cp[k][:, slot0 * CS:(slot0 + n) * CS]
                        dst = dst.rearrange("p (j c) -> p j c", j=n, c=CS)
                        with nc.allow_low_precision(
                            reason="fp16 col sums; 2e-2 tolerance"
                        ):
                            nc.vector.tensor_reduce(dst, src, axis=AXX,
                                                    op=ADD)

            # --------- phase A+C+D per s-chunk ---------------------------
            for m in range(Mh):
                rt = rtpool.tile([P, FW], fp16, tag="rt", name=f"rt{m}")
                ks = plan["overlap"][m]
                for t_idx in range(plan["n_tiles"]):
                    n0 = t_idx * PSW
                    nw = min(PSW, CPW - n0)
                    ps = pspool.tile([P, PSW], fp32, tag="ps",
                                     name=f"ps{m}_{t_idx}")
                    for i, k in enumerate(ks):
                        nc.tensor.matmul(
                            ps[:, :nw],
                            prT_sb[k][:, m * P:(m + 1) * P],
                            cp[k][:, n0:n0 + nw],
                            start=(i == 0),
                            stop=(i == len(ks) - 1),
                        )
                    for (L, n, slot0, lw0) in plan["c_pieces"][t_idx]:
                        src = ps[:, slot0 * CS - n0:(slot0 + n) * CS - n0]
                        src = src.rearrange("p (j c) -> p j c", j=n, c=CS)
                        src = src.unsqueeze(2).broadcast_to([P, n, L, CS])
                        dst = rt[:, lw0 * CS:(lw0 + n * L) * CS]
                        dst = dst.rearrange("p (j l c) -> p j l c",
                                            j=n, l=L, c=CS)
                        nc.vector.tensor_scalar_mul(dst, src, 1.0 / L)
                for (r0, n, L, j0) in plan["d_runs"][m]:
                    src = rt[j0:j0 + n, :].unsqueeze(1)
                    src = src.broadcast_to([n, L, FW])
                    dst = y[r0:r0 + n * L, :]
                    dst = dst.rearrange("(n l) f -> n l f", n=n, l=L)
                    nc.sync.dma_start(dst, src)

    nc.compile()
    nc.finalize()
    return nc


def _prep_host(input, h_mask, v_mask):
    """Returns (nc, in_maps, plan) ready for execution."""
    row_segs = _segments(h_mask)
    col_segs = _segments(v_mask)
    plan = _plan(row_segs, col_segs)
    nc = _build_program(row_segs, col_segs, plan)
    in_maps = _make_in_maps(input, row_segs, plan)
    return nc, in_maps, plan


def _make_in_maps(input, row_segs, plan):
    Mh = plan["Mh"]
    prT = np.zeros((H, Mh * P), dtype=np.float16)
    for m in range(Mh):
        chunk = plan["seg_perm"][m * P:(m + 1) * P]
        for j, s in enumerate(chunk):
            a, b = row_segs[s]
            prT[a:b, m * P + j] = np.float16(1.0 / (b - a))

    xp = np.asarray(input)[0].astype(np.float16)[:, plan["wperm"], :]
    in_maps = []
    for k in range(NCORES):
        xc = np.ascontiguousarray(xp[:, :, k * CS:(k + 1) * CS])
        in_maps.append({"x": xc.reshape(H, FW), "prT": prT})
    return in_maps


# stash for test.py introspection
LAST_RESULT = {}
_EXEC_CACHE = {}


def _make_executable(nc):
    """Build a reusable sharded jit callable for this program.

    Mirrors bass2jax.run_bass_via_pjrt's multi-core branch but keeps the
    jitted function so repeated calls skip retrace/recompile (and so the
    test harness can time steady-state executions).
    """
    import jax
    import concourse.mybir as mybir
    from concourse import bass2jax
    from jax.sharding import Mesh, PartitionSpec
    from jax.experimental.shard_map import shard_map

    bass2jax.install_neuronx_cc_hook()

    partition_name = (
        nc.partition_id_tensor.name if nc.partition_id_tensor else None
    )
    in_names, out_names, out_shapes, out_dtypes = [], [], [], []
    for alloc in nc.m.functions[0].allocations:
        if not isinstance(alloc, mybir.MemoryLocationSet):
            continue
        name = alloc.memorylocations[0].name
        if alloc.kind == "ExternalInput":
            if name != partition_name:
                in_names.append(name)
        elif alloc.kind == "ExternalOutput":
            out_names.append(name)
            out_shapes.append(tuple(alloc.tensor_shape))
            out_dtypes.append(mybir.dt.np(alloc.dtype))
    out_avals = tuple(
        jax.core.ShapedArray(s, d) for s, d in zip(out_shapes, out_dtypes)
    )
    n_params = len(in_names)
    n_outs = len(out_names)
    all_names = in_names + out_names
    if partition_name is not None:
        all_names = all_names + [partition_name]

    def _body(*args):
        operands = list(args)
        if partition_name is not None:
            operands.append(bass2jax.partition_id_tensor())
        outs = bass2jax._bass_exec_p.bind(
            *operands,
            out_avals=out_avals,
            in_names=tuple(all_names),
            out_names=tuple(out_names),
            lowering_input_output_aliases=(),
            sim_require_finite=True,
            sim_require_nnan=True,
            nc=nc,
        )
        return tuple(outs)

    devices = jax.devices()[:NCORES]
    mesh = Mesh(np.asarray(devices), ("core",))
    donate = tuple(range(n_params, n_params + n_outs))
    sharded = jax.jit(
        shard_map(
            _body,
            mesh=mesh,
            in_specs=(PartitionSpec("core"),) * (n_params + n_outs),
            out_specs=(PartitionSpec("core"),) * n_outs,
            check_rep=False,
        ),
        donate_argnums=donate,
        keep_unused=True,
    )

    def run(in_maps):
        concat_in = [
            np.concatenate([m[name] for m in in_maps], axis=0)
            for name in in_names
        ]
        concat_zeros = [
            np.zeros((NCORES * s[0], *s[1:]), d)
            for s, d in zip(out_shapes, out_dtypes)
        ]
        out_arrs = sharded(*concat_in, *concat_zeros)
        return [
            {
                name: np.asarray(out_arrs[i]).reshape(
                    NCORES, *out_shapes[i]
                )[c]
                for i, name in enumerate(out_names)
            }
            for c in range(NCORES)
        ]

    return run


def _get_run(input, h_mask, v_mask):
    key = (np.asarray(h_mask).tobytes(), np.asarray(v_mask).tobytes())
    if key not in _EXEC_CACHE:
        nc, in_maps, plan = _prep_host(
            np.asarray(input), np.asarray(h_mask), np.asarray(v_mask)
        )
        LAST_RESULT["nc"] = nc
        _EXEC_CACHE[key] = (_make_executable(nc), plan)
    else:
        row_segs = _segments(h_mask)
        plan = _EXEC_CACHE[key][1]
        in_maps = _make_in_maps(np.asarray(input), row_segs, plan)
    return _EXEC_CACHE[key][0], in_maps


def kernel(input, h_mask, v_mask):
    run, in_maps = _get_run(input, h_mask, v_mask)
    results = run(in_maps)
    LAST_RESULT["results"] = results

    key = (np.asarray(h_mask).tobytes(), np.asarray(v_mask).tobytes())
    plan = _EXEC_CACHE[key][1]
    # un-permute rows (class-grouped) and cols (class-sorted), upcast
    orig2dev = np.empty(H, dtype=np.int64)
    orig2dev[plan["dev_rows"]] = np.arange(H)
    winv = np.empty(W, dtype=np.int64)
    winv[plan["wperm"]] = np.arange(W)

    out = np.empty((H, W, C), dtype=np.float32)
    for k in range(NCORES):
        yk = results[k]["y"].reshape(H, W, CS)
        out[:, :, k * CS:(k + 1) * CS] = yk[orig2dev][:, winv]
    return out[None]


# revision 8
# speedup vs baseline: 31147.6167x; 1.1201x over previous
"""GridPoolingLayer kernel for Trainium2 (8 NeuronCores, Bass/Tile).

Semantics: the 1D binary masks partition H/W into maximal runs of constant
value; the layer replaces every grid cell with its mean (keep_size=True).
The op is separable: col-segment-mean along W, then row-segment-mean along
H, then broadcast back over each cell.

Device strategy per core (channels sharded 8 ways, 32 ch/core), fp16:

  B) col pooling   cp[k] = segment-sum_w(x chunk k)   -- DVE tensor_reduce
     along the free axis.  W is pre-permuted host-side so col segments of
     equal length are adjacent -> one reduce instruction per length class
     per load block.
  A) row pooling   pooled = P_r^T @ cp                -- PE matmul
     (contraction over H on partitions), P_r one-hot/len fp16 matrix
     precomputed host-side; row segments are permuted within each
     128-segment chunk so equal lengths are adjacent.
  C) col expand    rowtile[:, w] = pooled[:, seg(w)] / len_w -- DVE
     tensor_scalar_mul reading PSUM directly with a step-0 broadcast AP,
     one instruction per (length class x PSUM tile) piece.
  D) row expand    y rows = broadcast of pooled rows  -- DMA straight
     from SBUF with a step-0 source AP, ONE DMA per (s-chunk x row length
     class): output rows are written in class-grouped order.

The host un-permutes both axes (pure gathers) while unsharding and
upcasts fp16 -> fp32.  fp16 keeps HBM traffic at 16 MB in + 16 MB out
per core (vs 64 MB for fp32) and runs the PE at full 16-bit rate; the
2e-2 harness tolerance leaves ~40x margin over fp16 rounding noise.
"""

import math
import numpy as np

H, W, C = 512, 512, 256
NCORES = 8
CS = C // NCORES  # 32 channels per core
P = 128
FW = W * CS       # full row free size in elements (16384)
PSW = 512         # psum tile width (fp32 elems, one bank)
SLOTS_PER_TILE = PSW // CS  # 16 col segments per psum tile

TB = 128          # x load block target width (w units; 128 -> 1MB DMAs)
XIN_BUFS = 3
RT_BUFS = 2
PS_BUFS = 7


def _segments(mask):
    m = np.asarray(mask).ravel()
    change = np.nonzero(m[1:] != m[:-1])[0] + 1
    bounds = np.concatenate([[0], change, [len(m)]]).astype(np.int64)
    return [(int(bounds[i]), int(bounds[i + 1])) for i in range(len(bounds) - 1)]


def _plan(row_segs, col_segs):
    """Host-side geometry planning shared by program build + data prep."""
    from collections import defaultdict

    S_h, S_w = len(row_segs), len(col_segs)
    Mh = math.ceil(S_h / P)
    Kh = math.ceil(H / P)

    # ---- column side: class-sorted device order -------------------------
    by_len = defaultdict(list)
    for t, (u, v) in enumerate(col_segs):
        by_len[v - u].append(t)
    perm_cols = [t for L in sorted(by_len) for t in by_len[L]]

    wperm = np.empty(W, dtype=np.int64)   # dev w unit -> orig w
    wstart = np.empty(S_w + 1, dtype=np.int64)  # dev slot -> dev w unit
    off = 0
    for sl, t in enumerate(perm_cols):
        u, v = col_segs[t]
        wstart[sl] = off
        wperm[off:off + (v - u)] = np.arange(u, v)
        off += v - u
    wstart[S_w] = off
    assert off == W

    # class runs in device slot order: (L, n, lw0, slot0)
    col_runs = []
    sl = 0
    for L in sorted(by_len):
        n = len(by_len[L])
        col_runs.append((L, n, int(wstart[sl]), sl))
        sl += n

    # ---- x load blocks over the L>=2 classes, split at ~TB w units ------
    # (the L==1 class is DMAed straight into the col-pooled tile)
    load_blocks = []
    cur = {"w0": None, "wb": 0, "runs": []}
    for (L, n, lw0, slot0) in col_runs:
        if L == 1:
            continue
        i = 0
        while i < n:
            if cur["w0"] is None:
                cur["w0"] = lw0 + i * L
            room = max(1, (TB - cur["wb"]) // L)
            take = min(room, n - i)
            cur["runs"].append((L, take, cur["wb"], slot0 + i))
            cur["wb"] += take * L
            i += take
            if cur["wb"] >= TB:
                load_blocks.append(cur)
                cur = {"w0": None, "wb": 0, "runs": []}
    if cur["wb"]:
        load_blocks.append(cur)

    # ---- C pieces: class runs split at psum tile boundaries -------------
    # piece: (tile_idx, L, n, slot0, lw0)
    c_pieces = defaultdict(list)
    for (L, n, lw0, slot0) in col_runs:
        i = 0
        while i < n:
            t_idx = (slot0 + i) // SLOTS_PER_TILE
            room = (t_idx + 1) * SLOTS_PER_TILE - (slot0 + i)
            take = min(room, n - i)
            c_pieces[t_idx].append(
                (L, take, slot0 + i, int(wstart[slot0 + i]))
            )
            i += take

    n_tiles = math.ceil(S_w * CS / PSW)

    # ---- row side: class-sorted order within each s-chunk ---------------
    # dev_rows[r] = orig h; d_runs[m] = [(r0, n, L, j0)]
    dev_rows = np.empty(H, dtype=np.int64)
    d_runs = [[] for _ in range(Mh)]
    seg_perm = []  # global seg order: for m, class-sorted within chunk
    r0 = 0
    for m in range(Mh):
        chunk = list(range(m * P, min(S_h, (m + 1) * P)))
        chunk.sort(key=lambda s: (row_segs[s][1] - row_segs[s][0], s))
        j = 0
        while j < len(chunk):
            L = row_segs[chunk[j]][1] - row_segs[chunk[j]][0]
            n = 0
            while j + n < len(chunk) and (
                row_segs[chunk[j + n]][1] - row_segs[chunk[j + n]][0] == L
            ):
                n += 1
            d_runs[m].append((r0, n, L, j))
            for jj in range(n):
                a, b = row_segs[chunk[j + jj]]
                dev_rows[r0:r0 + L] = np.arange(a, b)
                r0 += L
            j += n
        seg_perm.extend(chunk)
    assert r0 == H

    # ---- row chunk overlap: which h-chunks feed each s-chunk ------------
    overlap = []
    for m in range(Mh):
        s_lo, s_hi = m * P, min(S_h, (m + 1) * P)
        h_lo = min(row_segs[s][0] for s in range(s_lo, s_hi))
        h_hi = max(row_segs[s][1] for s in range(s_lo, s_hi))
        overlap.append(
            [k for k in range(Kh) if k * P < h_hi and (k + 1) * P > h_lo]
        )

    return dict(
        S_h=S_h, S_w=S_w, Mh=Mh, Kh=Kh,
        wperm=wperm, col_runs=col_runs, load_blocks=load_blocks,
        c_pieces=c_pieces, n_tiles=n_tiles,
        dev_rows=dev_rows, d_runs=d_runs, seg_perm=seg_perm,
        overlap=overlap,
    )


def _build_program(row_segs, col_segs, plan):
    import concourse.bass as bass
    import concourse.mybir as mybir
    import concourse.tile as tile
    from concourse import bacc

    fp16 = mybir.dt.float16
    fp32 = mybir.dt.float32
    ADD = mybir.AluOpType.add
    AXX = mybir.AxisListType.X

    Mh, Kh = plan["Mh"], plan["Kh"]
    S_w = plan["S_w"]
    CPW = S_w * CS  # col-pooled row free size

    nc = bacc.Bacc()
    x = nc.dram_tensor("x", [H, FW], fp16, kind="ExternalInput")
    prT = nc.dram_tensor("prT", [H, Mh * P], fp16, kind="ExternalInput")
    y = nc.dram_tensor("y", [H, FW], fp16, kind="ExternalOutput")

    with tile.TileContext(nc) as tc:
        with (
            tc.tile_pool(name="consts", bufs=1) as consts,
            tc.tile_pool(name="xin", bufs=XIN_BUFS) as xin,
            tc.tile_pool(name="cp", bufs=1) as cppool,
            tc.tile_pool(name="rt", bufs=RT_BUFS) as rtpool,
            tc.tile_pool(name="ps", bufs=PS_BUFS, space="PSUM") as pspool,
            tc.tile_pool(name="warm", bufs=1, space="PSUM") as warmpool,
        ):
            # stationary pooling matrices, one [P, Mh*P] tile per h-chunk
            prT_sb = []
            for k in range(Kh):
                t = consts.tile([P, Mh * P], fp16, name=f"prT{k}")
                nc.sync.dma_start(t[:], prT[k * P:(k + 1) * P, :])
                prT_sb.append(t)

            # PE pre-touch of every prT tile: later matmuls then reach the
            # stationary operand without a DMA wait (keeps the LDWEIGHTS
            # sync-wait count within the ISA limit).
            ps_warm = warmpool.tile([1, PSW], fp32, name="ps_warm")
            for k in range(Kh):
                nc.tensor.matmul(
                    ps_warm[:1, :1],
                    prT_sb[k][:, :1],
                    prT_sb[k][:, :1],
                    start=True,
                    stop=True,
                )

            # ------------- phase B: load + col segment-sum ---------------
            # x element layout per class block: [j, c, l] (channel-major
            # inside each segment) so the reduce over l reads contiguous
            # fp16; the L==1 class is plain [j, c] and goes straight into
            # cp via DMA.  All loads ride the Activation HWDGE ring so
            # output stores keep the SP ring to themselves.
            cp = [
                cppool.tile([P, CPW], fp16, name=f"cp{k}") for k in range(Kh)
            ]
            one_run = next(
                ((L, n, lw0, slot0) for (L, n, lw0, slot0)
                 in plan["col_runs"] if L == 1), None
            )
            for k in range(Kh):
                if one_run is not None:
                    L, n, lw0, slot0 = one_run
                    nc.scalar.dma_start(
                        cp[k][:, slot0 * CS:(slot0 + n) * CS],
                        x[k * P:(k + 1) * P, lw0 * CS:(lw0 + n) * CS],
                    )
                for bi, blk in enumerate(plan["load_blocks"]):
                    wb = blk["wb"]
                    xt = xin.tile([P, wb * CS], fp16, tag="xt",
                                  name=f"xt{k}_{bi}")
                    nc.scalar.dma_start(
                        xt[:],
                        x[k * P:(k + 1) * P,
                          blk["w0"] * CS:(blk["w0"] + wb) * CS],
                    )
                    for (L, n, lw0, slot0) in blk["runs"]:
                        src = xt[:, lw0 * CS:(lw0 + n * L) * CS]
                        src = src.rearrange(
                            "p (j c l) -> p j c l", j=n, l=L, c=CS
                        )
                        dst = cp[k][:, slot0 * CS:(slot0 + n) * CS]
                        dst = dst.rearrange("p (j c) -> p j c", j=n, c=CS)
                        with nc.allow_low_precision(
                            reason="fp16 col sums; 2e-2 tolerance"
                        ):
                            nc.vector.tensor_reduce(dst, src, axis=AXX,
                                                    op=ADD)

            # --------- phase A+C+D per s-chunk ---------------------------
            for m in range(Mh):
                rt = rtpool.tile([P, FW], fp16, tag="rt", name=f"rt{m}")
                ks = plan["overlap"][m]
                for t_idx in range(plan["n_tiles"]):
                    n0 = t_idx * PSW
                    nw = min(PSW, CPW - n0)
                    ps = pspool.tile([P, PSW], fp32, tag="ps",
                                     name=f"ps{m}_{t_idx}")
                    for i, k in enumerate(ks):
                        nc.tensor.matmul(
                            ps[:, :nw],
                            prT_sb[k][:, m * P:(m + 1) * P],
                            cp[k][:, n0:n0 + nw],
                            start=(i == 0),
                            stop=(i == len(ks) - 1),
                        )
                    for (L, n, slot0, lw0) in plan["c_pieces"][t_idx]:
                        src = ps[:, slot0 * CS - n0:(slot0 + n) * CS - n0]
                        src = src.rearrange("p (j c) -> p j c", j=n, c=CS)
                        src = src.unsqueeze(2).broadcast_to([P, n, L, CS])
                        dst = rt[:, lw0 * CS:(lw0 + n * L) * CS]
                        dst = dst.rearrange("p (j l c) -> p j l c",
                                            j=n, l=L, c=CS)
                        nc.vector.tensor_scalar_mul(dst, src, 1.0 / L)
                for di, (r0, n, L, j0) in enumerate(plan["d_runs"][m]):
                    src = rt[j0:j0 + n, :].unsqueeze(1)
                    src = src.broadcast_to([n, L, FW])
                    dst = y[r0:r0 + n * L, :]
                    dst = dst.rearrange("(n l) f -> n l f", n=n, l=L)
                    eng = nc.sync if di % 2 == 0 else nc.gpsimd
                    eng.dma_start(dst, src)

    nc.compile()
    nc.finalize()
    return nc


def _prep_host(input, h_mask, v_mask):
    """Returns (nc, in_maps, plan) ready for execution."""
    row_segs = _segments(h_mask)
    col_segs = _segments(v_mask)
    plan = _plan(row_segs, col_segs)
    nc = _build_program(row_segs, col_segs, plan)
    in_maps = _make_in_maps(input, row_segs, plan)
    return nc, in_maps, plan


def _make_in_maps(input, row_segs, plan):
    Mh = plan["Mh"]
    prT = np.zeros((H, Mh * P), dtype=np.float16)
    for m in range(Mh):
        chunk = plan["seg_perm"][m * P:(m + 1) * P]
        for j, s in enumerate(chunk):
            a, b = row_segs[s]
            prT[a:b, m * P + j] = np.float16(1.0 / (b - a))

    # device x layout: class blocks in slot order; within a class block
    # each segment is [c, l] (channel-major) so the device reduce over l
    # is contiguous.  wperm[lw0:lw0+n*L] gives the class's orig w cols.
    xp16 = np.asarray(input)[0].astype(np.float16)  # [H, W, C]
    parts = []  # per class: [H, n, C, L]
    for (L, n, lw0, slot0) in plan["col_runs"]:
        cols = plan["wperm"][lw0:lw0 + n * L]
        blk = xp16[:, cols, :].reshape(H, n, L, C)
        parts.append(np.ascontiguousarray(blk.transpose(0, 1, 3, 2)))
    in_maps = []
    for k in range(NCORES):
        xc = np.concatenate(
            [p[:, :, k * CS:(k + 1) * CS, :].reshape(H, -1) for p in parts],
            axis=1,
        )
        in_maps.append({"x": np.ascontiguousarray(xc), "prT": prT})
    return in_maps


# stash for test.py introspection
LAST_RESULT = {}
_EXEC_CACHE = {}


def _make_executable(nc):
    """Build a reusable sharded jit callable for this program.

    Mirrors bass2jax.run_bass_via_pjrt's multi-core branch but keeps the
    jitted function so repeated calls skip retrace/recompile (and so the
    test harness can time steady-state executions).
    """
    import jax
    import concourse.mybir as mybir
    from concourse import bass2jax
    from jax.sharding import Mesh, PartitionSpec
    from jax.experimental.shard_map import shard_map

    bass2jax.install_neuronx_cc_hook()

    partition_name = (
        nc.partition_id_tensor.name if nc.partition_id_tensor else None
    )
    in_names, out_names, out_shapes, out_dtypes = [], [], [], []
    for alloc in nc.m.functions[0].allocations:
        if not isinstance(alloc, mybir.MemoryLocationSet):
            continue
        name = alloc.memorylocations[0].name
        if alloc.kind == "ExternalInput":
            if name != partition_name:
                in_names.append(name)
        elif alloc.kind == "ExternalOutput":
            out_names.append(name)
            out_shapes.append(tuple(alloc.tensor_shape))
            out_dtypes.append(mybir.dt.np(alloc.dtype))
    out_avals = tuple(
        jax.core.ShapedArray(s, d) for s, d in zip(out_shapes, out_dtypes)
    )
    n_params = len(in_names)
    n_outs = len(out_names)
    all_names = in_names + out_names
    if partition_name is not None:
        all_names = all_names + [partition_name]

    def _body(*args):
        operands = list(args)
        if partition_name is not None:
            operands.append(bass2jax.partition_id_tensor())
        outs = bass2jax._bass_exec_p.bind(
            *operands,
            out_avals=out_avals,
            in_names=tuple(all_names),
            out_names=tuple(out_names),
            lowering_input_output_aliases=(),
            sim_require_finite=True,
            sim_require_nnan=True,
            nc=nc,
        )
        return tuple(outs)

    devices = jax.devices()[:NCORES]
    mesh = Mesh(np.asarray(devices), ("core",))
    donate = tuple(range(n_params, n_params + n_outs))
    sharded = jax.jit(
        shard_map(
            _body,
            mesh=mesh,
            in_specs=(PartitionSpec("core"),) * (n_params + n_outs),
            out_specs=(PartitionSpec("core"),) * n_outs,
            check_rep=False,
        ),
        donate_argnums=donate,
        keep_unused=True,
    )

    def run(in_maps):
        concat_in = [
            np.concatenate([m[name] for m in in_maps], axis=0)
            for name in in_names
        ]
        concat_zeros = [
            np.zeros((NCORES * s[0], *s[1:]), d)
            for s, d in zip(out_shapes, out_dtypes)
        ]
        out_arrs = sharded(*concat_in, *concat_zeros)
        return [
            {
                name: np.asarray(out_arrs[i]).reshape(
                    NCORES, *out_shapes[i]
                )[c]
                for i, name in enumerate(out_names)
            }
            for c in range(NCORES)
        ]

    return run


def _get_run(input, h_mask, v_mask):
    key = (np.asarray(h_mask).tobytes(), np.asarray(v_mask).tobytes())
    if key not in _EXEC_CACHE:
        nc, in_maps, plan = _prep_host(
            np.asarray(input), np.asarray(h_mask), np.asarray(v_mask)
        )
        LAST_RESULT["nc"] = nc
        _EXEC_CACHE[key] = (_make_executable(nc), plan)
    else:
        row_segs = _segments(h_mask)
        plan = _EXEC_CACHE[key][1]
        in_maps = _make_in_maps(np.asarray(input), row_segs, plan)
    return _EXEC_CACHE[key][0], in_maps


def kernel(input, h_mask, v_mask):
    run, in_maps = _get_run(input, h_mask, v_mask)
    results = run(in_maps)
    LAST_RESULT["results"] = results

    key = (np.asarray(h_mask).tobytes(), np.asarray(v_mask).tobytes())
    plan = _EXEC_CACHE[key][1]
    # un-permute rows (class-grouped) and cols (class-sorted), upcast
    orig2dev = np.empty(H, dtype=np.int64)
    orig2dev[plan["dev_rows"]] = np.arange(H)
    winv = np.empty(W, dtype=np.int64)
    winv[plan["wperm"]] = np.arange(W)

    out = np.empty((H, W, C), dtype=np.float32)
    for k in range(NCORES):
        yk = results[k]["y"].reshape(H, W, CS)
        out[:, :, k * CS:(k + 1) * CS] = yk[orig2dev][:, winv]
    return out[None]


# revision 13
# speedup vs baseline: 56587.0526x; 1.8167x over previous
"""GridPoolingLayer kernel for Trainium2 (8 NeuronCores, Bass/Tile).

Semantics: the 1D binary masks partition H/W into maximal runs of constant
value; the layer replaces every grid cell with its mean (keep_size=True).
The op is separable: col-segment-mean along W, then row-segment-mean along
H, then broadcast back over each cell.

Device strategy per core (channels sharded 8 ways, 32 ch/core), fp16:

  B) col pooling   cp[k] = segment-sum_w(x chunk k)   -- DVE tensor_reduce
     along the free axis.  W is pre-permuted host-side so col segments of
     equal length are adjacent -> one reduce instruction per length class
     per load block.
  A) row pooling   pooled = P_r^T @ cp                -- PE matmul
     (contraction over H on partitions), P_r one-hot/len fp16 matrix
     precomputed host-side; row segments are permuted within each
     128-segment chunk so equal lengths are adjacent.
  C) col expand    rowtile[:, w] = pooled[:, seg(w)] / len_w -- DVE
     tensor_scalar_mul reading PSUM directly with a step-0 broadcast AP,
     one instruction per (length class x PSUM tile) piece.
  D) row expand    y rows = broadcast of pooled rows  -- DMA straight
     from SBUF with a step-0 source AP, ONE DMA per (s-chunk x row length
     class): output rows are written in class-grouped order.

The host un-permutes both axes (pure gathers) while unsharding and
upcasts fp16 -> fp32.  fp16 keeps HBM traffic at 16 MB in + 16 MB out
per core (vs 64 MB for fp32) and runs the PE at full 16-bit rate; the
2e-2 harness tolerance leaves ~40x margin over fp16 rounding noise.
"""

import math
import numpy as np

H, W, C = 512, 512, 256
NCORES = 8
CS = C // NCORES  # 32 channels per core
P = 128
FW = W * CS       # full row free size in elements (16384)
PSW = 512         # psum tile width (fp32 elems, one bank)
SLOTS_PER_TILE = PSW // CS  # 16 col segments per psum tile

TB = 128          # x load block target width (w units; 128 -> 1MB DMAs)
XIN_BUFS = 3
RT_BUFS = 2
PS_BUFS = 7


def _segments(mask):
    m = np.asarray(mask).ravel()
    change = np.nonzero(m[1:] != m[:-1])[0] + 1
    bounds = np.concatenate([[0], change, [len(m)]]).astype(np.int64)
    return [(int(bounds[i]), int(bounds[i + 1])) for i in range(len(bounds) - 1)]


def _plan(row_segs, col_segs):
    """Host-side geometry planning shared by program build + data prep."""
    from collections import defaultdict

    S_h, S_w = len(row_segs), len(col_segs)
    Mh = math.ceil(S_h / P)
    Kh = math.ceil(H / P)

    # ---- column side: class-sorted device order -------------------------
    by_len = defaultdict(list)
    for t, (u, v) in enumerate(col_segs):
        by_len[v - u].append(t)
    perm_cols = [t for L in sorted(by_len) for t in by_len[L]]

    wperm = np.empty(W, dtype=np.int64)   # dev w unit -> orig w
    wstart = np.empty(S_w + 1, dtype=np.int64)  # dev slot -> dev w unit
    off = 0
    for sl, t in enumerate(perm_cols):
        u, v = col_segs[t]
        wstart[sl] = off
        wperm[off:off + (v - u)] = np.arange(u, v)
        off += v - u
    wstart[S_w] = off
    assert off == W

    # class runs in device slot order: (L, n, lw0, slot0)
    col_runs = []
    sl = 0
    for L in sorted(by_len):
        n = len(by_len[L])
        col_runs.append((L, n, int(wstart[sl]), sl))
        sl += n

    # ---- x load blocks over the L>=2 classes, split at ~TB w units ------
    # (the L==1 class is DMAed straight into the col-pooled tile)
    load_blocks = []
    cur = {"w0": None, "wb": 0, "runs": []}
    for (L, n, lw0, slot0) in col_runs:
        if L == 1:
            continue
        i = 0
        while i < n:
            if cur["w0"] is None:
                cur["w0"] = lw0 + i * L
            room = max(1, (TB - cur["wb"]) // L)
            take = min(room, n - i)
            cur["runs"].append((L, take, cur["wb"], slot0 + i))
            cur["wb"] += take * L
            i += take
            if cur["wb"] >= TB:
                load_blocks.append(cur)
                cur = {"w0": None, "wb": 0, "runs": []}
    if cur["wb"]:
        load_blocks.append(cur)

    # ---- C pieces: class runs split at psum tile boundaries -------------
    # piece: (tile_idx, L, n, slot0, lw0)
    c_pieces = defaultdict(list)
    for (L, n, lw0, slot0) in col_runs:
        i = 0
        while i < n:
            t_idx = (slot0 + i) // SLOTS_PER_TILE
            room = (t_idx + 1) * SLOTS_PER_TILE - (slot0 + i)
            take = min(room, n - i)
            c_pieces[t_idx].append(
                (L, take, slot0 + i, int(wstart[slot0 + i]))
            )
            i += take

    n_tiles = math.ceil(S_w * CS / PSW)

    # ---- row side: class-sorted order within each s-chunk ---------------
    # dev_rows[r] = orig h; d_runs[m] = [(r0, n, L, j0)]
    dev_rows = np.empty(H, dtype=np.int64)
    d_runs = [[] for _ in range(Mh)]
    seg_perm = []  # global seg order: for m, class-sorted within chunk
    r0 = 0
    for m in range(Mh):
        chunk = list(range(m * P, min(S_h, (m + 1) * P)))
        chunk.sort(key=lambda s: (row_segs[s][1] - row_segs[s][0], s))
        j = 0
        while j < len(chunk):
            L = row_segs[chunk[j]][1] - row_segs[chunk[j]][0]
            n = 0
            while j + n < len(chunk) and (
                row_segs[chunk[j + n]][1] - row_segs[chunk[j + n]][0] == L
            ):
                n += 1
            d_runs[m].append((r0, n, L, j))
            for jj in range(n):
                a, b = row_segs[chunk[j + jj]]
                dev_rows[r0:r0 + L] = np.arange(a, b)
                r0 += L
            j += n
        seg_perm.extend(chunk)
    assert r0 == H

    # ---- row chunk overlap: which h-chunks feed each s-chunk ------------
    overlap = []
    for m in range(Mh):
        s_lo, s_hi = m * P, min(S_h, (m + 1) * P)
        h_lo = min(row_segs[s][0] for s in range(s_lo, s_hi))
        h_hi = max(row_segs[s][1] for s in range(s_lo, s_hi))
        overlap.append(
            [k for k in range(Kh) if k * P < h_hi and (k + 1) * P > h_lo]
        )

    # ---- output windows: <=128 consecutive device rows, one s-chunk -----
    windows = []
    for m in range(Mh):
        R0 = d_runs[m][0][0]
        R1 = d_runs[m][-1][0] + d_runs[m][-1][1] * d_runs[m][-1][2]
        a = R0
        while a < R1:
            windows.append((m, a, min(P, R1 - a)))
            a += P

    return dict(
        S_h=S_h, S_w=S_w, Mh=Mh, Kh=Kh,
        wperm=wperm, col_runs=col_runs, load_blocks=load_blocks,
        c_pieces=c_pieces, n_tiles=n_tiles,
        dev_rows=dev_rows, d_runs=d_runs, seg_perm=seg_perm,
        overlap=overlap, windows=windows,
    )


def _build_program(row_segs, col_segs, plan):
    import concourse.bass as bass
    import concourse.mybir as mybir
    import concourse.tile as tile
    from concourse import bacc

    fp16 = mybir.dt.float16
    fp32 = mybir.dt.float32
    ADD = mybir.AluOpType.add
    AXX = mybir.AxisListType.X

    Mh, Kh = plan["Mh"], plan["Kh"]
    S_w = plan["S_w"]
    CPW = S_w * CS  # col-pooled row free size

    COPY = mybir.ActivationFunctionType.Copy

    nc = bacc.Bacc()
    x = nc.dram_tensor("x", [H, FW], fp16, kind="ExternalInput")
    prT = nc.dram_tensor("prT", [H, Mh * P], fp16, kind="ExternalInput")
    rE = nc.dram_tensor("rE", [Mh * P, H], fp16, kind="ExternalInput")
    y = nc.dram_tensor("y", [H, FW], fp16, kind="ExternalOutput")

    with tile.TileContext(nc) as tc:
        with (
            tc.tile_pool(name="consts", bufs=1) as consts,
            tc.tile_pool(name="xin", bufs=XIN_BUFS) as xin,
            tc.tile_pool(name="cp", bufs=1) as cppool,
            tc.tile_pool(name="rt", bufs=1) as rtpool,
            tc.tile_pool(name="ot", bufs=2) as otpool,
            tc.tile_pool(name="ps", bufs=3, space="PSUM") as pspool,
            tc.tile_pool(name="ps2", bufs=2, space="PSUM") as ps2pool,
            tc.tile_pool(name="warm", bufs=1, space="PSUM") as warmpool,
        ):
            # stationary matrices: row-pool prT per h-chunk, row-expand
            # one-hot rE per s-chunk
            prT_sb = []
            for k in range(Kh):
                t = consts.tile([P, Mh * P], fp16, name=f"prT{k}")
                nc.sync.dma_start(t[:], prT[k * P:(k + 1) * P, :])
                prT_sb.append(t)
            rE_sb = []
            for m in range(Mh):
                t = consts.tile([P, H], fp16, name=f"rE{m}")
                nc.sync.dma_start(t[:], rE[m * P:(m + 1) * P, :])
                rE_sb.append(t)

            # PE pre-touch of every stationary tile: later matmuls then
            # reach the operand without a DMA wait (keeps the LDWEIGHTS
            # sync-wait count within the ISA limit).
            ps_warm = warmpool.tile([1, PSW], fp32, name="ps_warm")
            for t in prT_sb + rE_sb:
                nc.tensor.matmul(
                    ps_warm[:1, :1],
                    t[:, :1],
                    t[:, :1],
                    start=True,
                    stop=True,
                )

            # ------------- phase B: load + col segment-sum ---------------
            # x element layout per class block: [j, c, l] (channel-major
            # inside each segment) so the reduce over l reads contiguous
            # fp16; the L==1 class is plain [j, c] and goes straight into
            # cp via DMA.  All loads ride the Activation HWDGE ring so
            # output stores keep the SP ring to themselves.
            cp = [
                cppool.tile([P, CPW], fp16, name=f"cp{k}") for k in range(Kh)
            ]
            one_run = next(
                ((L, n, lw0, slot0) for (L, n, lw0, slot0)
                 in plan["col_runs"] if L == 1), None
            )
            for k in range(Kh):
                if one_run is not None:
                    L, n, lw0, slot0 = one_run
                    nc.scalar.dma_start(
                        cp[k][:, slot0 * CS:(slot0 + n) * CS],
                        x[k * P:(k + 1) * P, lw0 * CS:(lw0 + n) * CS],
                    )
                for bi, blk in enumerate(plan["load_blocks"]):
                    wb = blk["wb"]
                    xt = xin.tile([P, wb * CS], fp16, tag="xt",
                                  name=f"xt{k}_{bi}")
                    nc.scalar.dma_start(
                        xt[:],
                        x[k * P:(k + 1) * P,
                          blk["w0"] * CS:(blk["w0"] + wb) * CS],
                    )
                    for (L, n, lw0, slot0) in blk["runs"]:
                        src = xt[:, lw0 * CS:(lw0 + n * L) * CS]
                        src = src.rearrange(
                            "p (j c l) -> p j c l", j=n, l=L, c=CS
                        )
                        dst = cp[k][:, slot0 * CS:(slot0 + n) * CS]
                        dst = dst.rearrange("p (j c) -> p j c", j=n, c=CS)
                        with nc.allow_low_precision(
                            reason="fp16 col sums; 2e-2 tolerance"
                        ):
                            nc.vector.tensor_reduce(dst, src, axis=AXX,
                                                    op=ADD)

            # --------- phase A+C+D per s-chunk ---------------------------
            # A: pool rows into psum; C: col-expand psum -> rt (fp16, full
            # width); D: row-expand rt via one-hot PE matmul into 128-row
            # output windows and store each window with one clean 2D DMA
            # (the PE reads rt across all 128 partitions in parallel --
            # a DMA broadcast source would be pinned to single SBUF ports).
            CHUNK = 2 * PSW  # psum2 tile: two banks, one copy instr
            wi_all = 0
            for m in range(Mh):
                rt = rtpool.tile([P, FW], fp16, tag="rt", name=f"rt{m}")
                ks = plan["overlap"][m]
                for t_idx in range(plan["n_tiles"]):
                    n0 = t_idx * PSW
                    nw = min(PSW, CPW - n0)
                    ps = pspool.tile([P, PSW], fp32, tag="ps",
                                     name=f"ps{m}_{t_idx}")
                    for i, k in enumerate(ks):
                        nc.tensor.matmul(
                            ps[:, :nw],
                            prT_sb[k][:, m * P:(m + 1) * P],
                            cp[k][:, n0:n0 + nw],
                            start=(i == 0),
                            stop=(i == len(ks) - 1),
                        )
                    for (L, n, slot0, lw0) in plan["c_pieces"][t_idx]:
                        src = ps[:, slot0 * CS - n0:(slot0 + n) * CS - n0]
                        src = src.rearrange("p (j c) -> p j c", j=n, c=CS)
                        src = src.unsqueeze(2).broadcast_to([P, n, L, CS])
                        dst = rt[:, lw0 * CS:(lw0 + n * L) * CS]
                        dst = dst.rearrange("p (j l c) -> p j l c",
                                            j=n, l=L, c=CS)
                        nc.vector.tensor_scalar_mul(dst, src, 1.0 / L)
                for (wm, a, wlen) in plan["windows"]:
                    if wm != m:
                        continue
                    ot = otpool.tile([P, FW], fp16, tag="ot",
                                     name=f"ot{m}_{a}")
                    for ci, c0 in enumerate(range(0, FW, CHUNK)):
                        ps2 = ps2pool.tile([P, CHUNK], fp32, tag="ps2",
                                           name=f"ps2_{m}_{a}_{c0}")
                        for half in range(2):
                            nc.tensor.matmul(
                                ps2[:wlen, half * PSW:(half + 1) * PSW],
                                rE_sb[m][:, a:a + wlen],
                                rt[:, c0 + half * PSW:c0 + (half + 1) * PSW],
                                start=True,
                                stop=True,
                            )
                        if ci % 2 == 0:
                            nc.scalar.activation(
                                ot[:wlen, c0:c0 + CHUNK],
                                ps2[:wlen, :],
                                COPY,
                            )
                        else:
                            nc.vector.tensor_scalar_mul(
                                ot[:wlen, c0:c0 + CHUNK],
                                ps2[:wlen, :],
                                1.0,
                            )
                    eng = nc.sync if wi_all % 2 == 0 else nc.gpsimd
                    eng.dma_start(y[a:a + wlen, :], ot[:wlen, :])
                    wi_all += 1

    nc.compile()
    nc.finalize()
    return nc


def _prep_host(input, h_mask, v_mask):
    """Returns (nc, in_maps, plan) ready for execution."""
    row_segs = _segments(h_mask)
    col_segs = _segments(v_mask)
    plan = _plan(row_segs, col_segs)
    nc = _build_program(row_segs, col_segs, plan)
    in_maps = _make_in_maps(input, row_segs, plan)
    return nc, in_maps, plan


def _make_in_maps(input, row_segs, plan):
    Mh = plan["Mh"]
    prT = np.zeros((H, Mh * P), dtype=np.float16)
    for m in range(Mh):
        chunk = plan["seg_perm"][m * P:(m + 1) * P]
        for j, s in enumerate(chunk):
            a, b = row_segs[s]
            prT[a:b, m * P + j] = np.float16(1.0 / (b - a))

    # device x layout: class blocks in slot order; within a class block
    # each segment is [c, l] (channel-major) so the device reduce over l
    # is contiguous.  wperm[lw0:lw0+n*L] gives the class's orig w cols.
    xp16 = np.asarray(input)[0].astype(np.float16)  # [H, W, C]
    parts = []  # per class: [H, n, C, L]
    for (L, n, lw0, slot0) in plan["col_runs"]:
        cols = plan["wperm"][lw0:lw0 + n * L]
        blk = xp16[:, cols, :].reshape(H, n, L, C)
        parts.append(np.ascontiguousarray(blk.transpose(0, 1, 3, 2)))
    # row-expansion one-hot: rE[m*P + j, r] = 1 iff device row r belongs
    # to the j-th (class-sorted) segment of s-chunk m
    Mh = plan["Mh"]
    rE = np.zeros((Mh * P, H), dtype=np.float16)
    for m in range(Mh):
        for (r0, n, L, j0) in plan["d_runs"][m]:
            for jj in range(n):
                rE[m * P + j0 + jj, r0 + jj * L:r0 + (jj + 1) * L] = 1.0

    in_maps = []
    for k in range(NCORES):
        xc = np.concatenate(
            [p[:, :, k * CS:(k + 1) * CS, :].reshape(H, -1) for p in parts],
            axis=1,
        )
        in_maps.append(
            {"x": np.ascontiguousarray(xc), "prT": prT, "rE": rE}
        )
    return in_maps


# stash for test.py introspection
LAST_RESULT = {}
_EXEC_CACHE = {}


def _make_executable(nc):
    """Build a reusable sharded jit callable for this program.

    Mirrors bass2jax.run_bass_via_pjrt's multi-core branch but keeps the
    jitted function so repeated calls skip retrace/recompile (and so the
    test harness can time steady-state executions).
    """
    import jax
    import concourse.mybir as mybir
    from concourse import bass2jax
    from jax.sharding import Mesh, PartitionSpec
    from jax.experimental.shard_map import shard_map

    bass2jax.install_neuronx_cc_hook()

    partition_name = (
        nc.partition_id_tensor.name if nc.partition_id_tensor else None
    )
    in_names, out_names, out_shapes, out_dtypes = [], [], [], []
    for alloc in nc.m.functions[0].allocations:
        if not isinstance(alloc, mybir.MemoryLocationSet):
            continue
        name = alloc.memorylocations[0].name
        if alloc.kind == "ExternalInput":
            if name != partition_name:
                in_names.append(name)
        elif alloc.kind == "ExternalOutput":
            out_names.append(name)
            out_shapes.append(tuple(alloc.tensor_shape))
            out_dtypes.append(mybir.dt.np(alloc.dtype))
    out_avals = tuple(
        jax.core.ShapedArray(s, d) for s, d in zip(out_shapes, out_dtypes)
    )
    n_params = len(in_names)
    n_outs = len(out_names)
    all_names = in_names + out_names
    if partition_name is not None:
        all_names = all_names + [partition_name]

    def _body(*args):
        operands = list(args)
        if partition_name is not None:
            operands.append(bass2jax.partition_id_tensor())
        outs = bass2jax._bass_exec_p.bind(
            *operands,
            out_avals=out_avals,
            in_names=tuple(all_names),
            out_names=tuple(out_names),
            lowering_input_output_aliases=(),
            sim_require_finite=True,
            sim_require_nnan=True,
            nc=nc,
        )
        return tuple(outs)

    devices = jax.devices()[:NCORES]
    mesh = Mesh(np.asarray(devices), ("core",))
    donate = tuple(range(n_params, n_params + n_outs))
    sharded = jax.jit(
        shard_map(
            _body,
            mesh=mesh,
            in_specs=(PartitionSpec("core"),) * (n_params + n_outs),
            out_specs=(PartitionSpec("core"),) * n_outs,
            check_rep=False,
        ),
        donate_argnums=donate,
        keep_unused=True,
    )

    def run(in_maps):
        concat_in = [
            np.concatenate([m[name] for m in in_maps], axis=0)
            for name in in_names
        ]
        concat_zeros = [
            np.zeros((NCORES * s[0], *s[1:]), d)
            for s, d in zip(out_shapes, out_dtypes)
        ]
        out_arrs = sharded(*concat_in, *concat_zeros)
        return [
            {
                name: np.asarray(out_arrs[i]).reshape(
                    NCORES, *out_shapes[i]
                )[c]
                for i, name in enumerate(out_names)
            }
            for c in range(NCORES)
        ]

    return run


def _get_run(input, h_mask, v_mask):
    key = (np.asarray(h_mask).tobytes(), np.asarray(v_mask).tobytes())
    if key not in _EXEC_CACHE:
        nc, in_maps, plan = _prep_host(
            np.asarray(input), np.asarray(h_mask), np.asarray(v_mask)
        )
        LAST_RESULT["nc"] = nc
        _EXEC_CACHE[key] = (_make_executable(nc), plan)
    else:
        row_segs = _segments(h_mask)
        plan = _EXEC_CACHE[key][1]
        in_maps = _make_in_maps(np.asarray(input), row_segs, plan)
    return _EXEC_CACHE[key][0], in_maps


def kernel(input, h_mask, v_mask):
    run, in_maps = _get_run(input, h_mask, v_mask)
    results = run(in_maps)
    LAST_RESULT["results"] = results

    key = (np.asarray(h_mask).tobytes(), np.asarray(v_mask).tobytes())
    plan = _EXEC_CACHE[key][1]
    # un-permute rows (class-grouped) and cols (class-sorted), upcast
    orig2dev = np.empty(H, dtype=np.int64)
    orig2dev[plan["dev_rows"]] = np.arange(H)
    winv = np.empty(W, dtype=np.int64)
    winv[plan["wperm"]] = np.arange(W)

    out = np.empty((H, W, C), dtype=np.float32)
    for k in range(NCORES):
        yk = results[k]["y"].reshape(H, W, CS)
        out[:, :, k * CS:(k + 1) * CS] = yk[orig2dev][:, winv]
    return out[None]


# revision 19
# speedup vs baseline: 67635.8098x; 1.1953x over previous
"""GridPoolingLayer kernel for Trainium2 (8 NeuronCores, Bass/Tile).

Semantics: the 1D binary masks partition H/W into maximal runs of constant
value; the layer replaces every grid cell with its mean (keep_size=True).
The op is separable: col-segment-mean along W, then row-segment-mean along
H, then broadcast back over each cell.

Device strategy per core (channels sharded 8 ways, 32 ch/core), fp16:

  B) col pooling   cp[k] = segment-sum_w(x chunk k)   -- DVE tensor_reduce
     along the free axis.  W is pre-permuted host-side so col segments of
     equal length are adjacent -> one reduce instruction per length class
     per load block.
  A) row pooling   pooled = P_r^T @ cp                -- PE matmul
     (contraction over H on partitions), P_r one-hot/len fp16 matrix
     precomputed host-side; row segments are permuted within each
     128-segment chunk so equal lengths are adjacent.
  C) col expand    rowtile[:, w] = pooled[:, seg(w)] / len_w -- DVE
     tensor_scalar_mul reading PSUM directly with a step-0 broadcast AP,
     one instruction per (length class x PSUM tile) piece.
  D) row expand    y rows = broadcast of pooled rows  -- DMA straight
     from SBUF with a step-0 source AP, ONE DMA per (s-chunk x row length
     class): output rows are written in class-grouped order.

The host un-permutes both axes (pure gathers) while unsharding and
upcasts fp16 -> fp32.  fp16 keeps HBM traffic at 16 MB in + 16 MB out
per core (vs 64 MB for fp32) and runs the PE at full 16-bit rate; the
2e-2 harness tolerance leaves ~40x margin over fp16 rounding noise.
"""

import math
import numpy as np

H, W, C = 512, 512, 256
NCORES = 8
CS = C // NCORES  # 32 channels per core
P = 128
FW = W * CS       # full row free size in elements (16384)
PSW = 512         # psum tile width (fp32 elems, one bank)
SLOTS_PER_TILE = PSW // CS  # 16 col segments per psum tile

TB = 128          # x load block target width (w units; 128 -> 1MB DMAs)
XIN_BUFS = 3
CHUNK2 = 1024     # psum2 tile width (fp32 elems, two banks)


def _segments(mask):
    m = np.asarray(mask).ravel()
    change = np.nonzero(m[1:] != m[:-1])[0] + 1
    bounds = np.concatenate([[0], change, [len(m)]]).astype(np.int64)
    return [(int(bounds[i]), int(bounds[i + 1])) for i in range(len(bounds) - 1)]


def _plan(row_segs, col_segs):
    """Host-side geometry planning shared by program build + data prep."""
    from collections import defaultdict

    S_h, S_w = len(row_segs), len(col_segs)
    Mh = math.ceil(S_h / P)
    Kh = math.ceil(H / P)

    # ---- column side: class-sorted device order -------------------------
    by_len = defaultdict(list)
    for t, (u, v) in enumerate(col_segs):
        by_len[v - u].append(t)
    perm_cols = [t for L in sorted(by_len) for t in by_len[L]]

    wperm = np.empty(W, dtype=np.int64)   # dev w unit -> orig w
    wstart = np.empty(S_w + 1, dtype=np.int64)  # dev slot -> dev w unit
    off = 0
    for sl, t in enumerate(perm_cols):
        u, v = col_segs[t]
        wstart[sl] = off
        wperm[off:off + (v - u)] = np.arange(u, v)
        off += v - u
    wstart[S_w] = off
    assert off == W

    # class runs in device slot order: (L, n, lw0, slot0)
    col_runs = []
    sl = 0
    for L in sorted(by_len):
        n = len(by_len[L])
        col_runs.append((L, n, int(wstart[sl]), sl))
        sl += n

    # ---- x load blocks over the L>=2 classes, split at ~TB w units ------
    # (the L==1 class is DMAed straight into the col-pooled tile)
    load_blocks = []
    cur = {"w0": None, "wb": 0, "runs": []}
    for (L, n, lw0, slot0) in col_runs:
        if L == 1:
            continue
        i = 0
        while i < n:
            if cur["w0"] is None:
                cur["w0"] = lw0 + i * L
            room = max(1, (TB - cur["wb"]) // L)
            take = min(room, n - i)
            cur["runs"].append((L, take, cur["wb"], slot0 + i))
            cur["wb"] += take * L
            i += take
            if cur["wb"] >= TB:
                load_blocks.append(cur)
                cur = {"w0": None, "wb": 0, "runs": []}
    if cur["wb"]:
        load_blocks.append(cur)

    # ---- expansion pieces: class runs split at psum2 chunk boundaries ---
    # chunk = CHUNK2 fp32 elems (CHUNK2//CS slots); piece: (L, n, slot0, lw0)
    slots_per_chunk = CHUNK2 // CS
    exp_pieces = defaultdict(list)
    for (L, n, lw0, slot0) in col_runs:
        i = 0
        while i < n:
            ch = (slot0 + i) // slots_per_chunk
            room = (ch + 1) * slots_per_chunk - (slot0 + i)
            take = min(room, n - i)
            exp_pieces[ch].append(
                (L, take, slot0 + i, int(wstart[slot0 + i]))
            )
            i += take

    n_tiles = math.ceil(S_w * CS / PSW)
    n_chunks = math.ceil(S_w * CS / CHUNK2)

    # ---- row side: class-sorted order within each s-chunk ---------------
    # dev_rows[r] = orig h; d_runs[m] = [(r0, n, L, j0)]
    dev_rows = np.empty(H, dtype=np.int64)
    d_runs = [[] for _ in range(Mh)]
    seg_perm = []  # global seg order: for m, class-sorted within chunk
    r0 = 0
    for m in range(Mh):
        chunk = list(range(m * P, min(S_h, (m + 1) * P)))
        chunk.sort(key=lambda s: (row_segs[s][1] - row_segs[s][0], s))
        j = 0
        while j < len(chunk):
            L = row_segs[chunk[j]][1] - row_segs[chunk[j]][0]
            n = 0
            while j + n < len(chunk) and (
                row_segs[chunk[j + n]][1] - row_segs[chunk[j + n]][0] == L
            ):
                n += 1
            d_runs[m].append((r0, n, L, j))
            for jj in range(n):
                a, b = row_segs[chunk[j + jj]]
                dev_rows[r0:r0 + L] = np.arange(a, b)
                r0 += L
            j += n
        seg_perm.extend(chunk)
    assert r0 == H

    # ---- row chunk overlap: which h-chunks feed each s-chunk ------------
    overlap = []
    for m in range(Mh):
        s_lo, s_hi = m * P, min(S_h, (m + 1) * P)
        h_lo = min(row_segs[s][0] for s in range(s_lo, s_hi))
        h_hi = max(row_segs[s][1] for s in range(s_lo, s_hi))
        overlap.append(
            [k for k in range(Kh) if k * P < h_hi and (k + 1) * P > h_lo]
        )

    # ---- output windows: <=128 consecutive device rows, one s-chunk -----
    windows = []
    for m in range(Mh):
        R0 = d_runs[m][0][0]
        R1 = d_runs[m][-1][0] + d_runs[m][-1][1] * d_runs[m][-1][2]
        a = R0
        while a < R1:
            windows.append((m, a, min(P, R1 - a)))
            a += P

    return dict(
        S_h=S_h, S_w=S_w, Mh=Mh, Kh=Kh,
        wperm=wperm, col_runs=col_runs, load_blocks=load_blocks,
        exp_pieces=exp_pieces, n_tiles=n_tiles, n_chunks=n_chunks,
        dev_rows=dev_rows, d_runs=d_runs, seg_perm=seg_perm,
        overlap=overlap, windows=windows,
    )


def _build_program(row_segs, col_segs, plan):
    import concourse.bass as bass
    import concourse.mybir as mybir
    import concourse.tile as tile
    from concourse import bacc

    fp16 = mybir.dt.float16
    fp32 = mybir.dt.float32
    ADD = mybir.AluOpType.add
    AXX = mybir.AxisListType.X

    Mh, Kh = plan["Mh"], plan["Kh"]
    S_w = plan["S_w"]
    CPW = S_w * CS  # col-pooled row free size

    COPY = mybir.ActivationFunctionType.Copy

    nc = bacc.Bacc()
    x = nc.dram_tensor("x", [H, FW], fp16, kind="ExternalInput")
    prT = nc.dram_tensor("prT", [H, Mh * P], fp16, kind="ExternalInput")
    rE = nc.dram_tensor("rE", [Mh * P, H], fp16, kind="ExternalInput")
    y = nc.dram_tensor("y", [H, FW], fp16, kind="ExternalOutput")

    with tile.TileContext(nc) as tc:
        with (
            tc.tile_pool(name="consts", bufs=1) as consts,
            tc.tile_pool(name="xin", bufs=XIN_BUFS) as xin,
            tc.tile_pool(name="cp", bufs=1) as cppool,
            tc.tile_pool(name="pld", bufs=2) as pldpool,
            tc.tile_pool(name="ot", bufs=2) as otpool,
            tc.tile_pool(name="ps", bufs=2, space="PSUM") as pspool,
            tc.tile_pool(name="ps2", bufs=2, space="PSUM") as ps2pool,
            tc.tile_pool(name="warm", bufs=1, space="PSUM") as warmpool,
        ):
            # stationary matrices: row-pool prT per h-chunk, row-expand
            # one-hot rE per s-chunk
            prT_sb = []
            for k in range(Kh):
                t = consts.tile([P, Mh * P], fp16, name=f"prT{k}")
                nc.sync.dma_start(t[:], prT[k * P:(k + 1) * P, :])
                prT_sb.append(t)
            rE_sb = []
            for m in range(Mh):
                t = consts.tile([P, H], fp16, name=f"rE{m}")
                nc.sync.dma_start(t[:], rE[m * P:(m + 1) * P, :])
                rE_sb.append(t)

            # PE pre-touch of every stationary tile: later matmuls then
            # reach the operand without a DMA wait (keeps the LDWEIGHTS
            # sync-wait count within the ISA limit).
            ps_warm = warmpool.tile([1, PSW], fp32, name="ps_warm")
            for t in prT_sb + rE_sb:
                nc.tensor.matmul(
                    ps_warm[:1, :1],
                    t[:, :1],
                    t[:, :1],
                    start=True,
                    stop=True,
                )

            # ------------- phase B: load + col segment-sum ---------------
            # x element layout per class block: [j, c, l] (channel-major
            # inside each segment) so the reduce over l reads contiguous
            # fp16; the L==1 class is plain [j, c] and goes straight into
            # cp via DMA.  All loads ride the Activation HWDGE ring so
            # output stores keep the SP ring to themselves.
            cp = [
                cppool.tile([P, CPW], fp16, name=f"cp{k}") for k in range(Kh)
            ]
            one_run = next(
                ((L, n, lw0, slot0) for (L, n, lw0, slot0)
                 in plan["col_runs"] if L == 1), None
            )
            for k in range(Kh):
                if one_run is not None:
                    L, n, lw0, slot0 = one_run
                    nc.scalar.dma_start(
                        cp[k][:, slot0 * CS:(slot0 + n) * CS],
                        x[k * P:(k + 1) * P, lw0 * CS:(lw0 + n) * CS],
                    )
                for bi, blk in enumerate(plan["load_blocks"]):
                    wb = blk["wb"]
                    xt = xin.tile([P, wb * CS], fp16, tag="xt",
                                  name=f"xt{k}_{bi}")
                    nc.scalar.dma_start(
                        xt[:],
                        x[k * P:(k + 1) * P,
                          blk["w0"] * CS:(blk["w0"] + wb) * CS],
                    )
                    for (L, n, lw0, slot0) in blk["runs"]:
                        src = xt[:, lw0 * CS:(lw0 + n * L) * CS]
                        src = src.rearrange(
                            "p (j c l) -> p j c l", j=n, l=L, c=CS
                        )
                        dst = cp[k][:, slot0 * CS:(slot0 + n) * CS]
                        dst = dst.rearrange("p (j c) -> p j c", j=n, c=CS)
                        with nc.allow_low_precision(
                            reason="fp16 col sums; 2e-2 tolerance"
                        ):
                            nc.vector.tensor_reduce(dst, src, axis=AXX,
                                                    op=ADD)

            # --------- phases A+D+C per s-chunk --------------------------
            # A: pool rows into psum1, copy to pooled (fp16, narrow).
            # D: row-expand the NARROW pooled tensor via one-hot PE matmul
            #    into 128-row output windows (PE reads pooled across all
            #    partitions in parallel -- a DMA broadcast source would be
            #    pinned to single SBUF ports, and expanding before the col
            #    expand halves the PE work and keeps its stream dense).
            # C: col-expand psum2 -> ot while downcasting, spread across
            #    Vector/GpSimd/Scalar.  One clean 2D store per window.
            exp_rr = 0
            for m in range(Mh):
                pld = pldpool.tile([P, CPW], fp16, tag="pld",
                                   name=f"pld{m}")
                ks = plan["overlap"][m]
                for t_idx in range(plan["n_tiles"]):
                    n0 = t_idx * PSW
                    nw = min(PSW, CPW - n0)
                    ps = pspool.tile([P, PSW], fp32, tag="ps",
                                     name=f"ps{m}_{t_idx}")
                    for i, k in enumerate(ks):
                        nc.tensor.matmul(
                            ps[:, :nw],
                            prT_sb[k][:, m * P:(m + 1) * P],
                            cp[k][:, n0:n0 + nw],
                            start=(i == 0),
                            stop=(i == len(ks) - 1),
                        )
                    nc.scalar.activation(
                        pld[:, n0:n0 + nw], ps[:, :nw], COPY
                    )
                for (wm, a, wlen) in plan["windows"]:
                    if wm != m:
                        continue
                    ot = otpool.tile([P, FW], fp16, tag="ot",
                                     name=f"ot{m}_{a}")
                    for ci in range(plan["n_chunks"]):
                        c0 = ci * CHUNK2
                        cw = min(CHUNK2, CPW - c0)
                        ps2 = ps2pool.tile([P, CHUNK2], fp32, tag="ps2",
                                           name=f"ps2_{m}_{a}_{ci}")
                        for h0 in range(0, cw, PSW):
                            hw = min(PSW, cw - h0)
                            nc.tensor.matmul(
                                ps2[:wlen, h0:h0 + hw],
                                rE_sb[m][:, a:a + wlen],
                                pld[:, c0 + h0:c0 + h0 + hw],
                                start=True,
                                stop=True,
                            )
                        for (L, n, slot0, lw0) in plan["exp_pieces"][ci]:
                            src = ps2[:wlen,
                                      slot0 * CS - c0:(slot0 + n) * CS - c0]
                            src = src.rearrange("p (j c) -> p j c",
                                                j=n, c=CS)
                            src = src.unsqueeze(2)
                            src = src.broadcast_to([wlen, n, L, CS])
                            dst = ot[:wlen, lw0 * CS:(lw0 + n * L) * CS]
                            dst = dst.rearrange("p (j l c) -> p j l c",
                                                j=n, l=L, c=CS)
                            if exp_rr % 2 == 0:
                                nc.vector.tensor_scalar_mul(dst, src,
                                                            1.0 / L)
                            else:
                                nc.scalar.activation(dst, src, COPY,
                                                     scale=1.0 / L)
                            exp_rr += 1
                    nc.sync.dma_start(y[a:a + wlen, :], ot[:wlen, :])

    nc.compile()
    nc.finalize()
    return nc


def _prep_host(input, h_mask, v_mask):
    """Returns (nc, in_maps, plan) ready for execution."""
    row_segs = _segments(h_mask)
    col_segs = _segments(v_mask)
    plan = _plan(row_segs, col_segs)
    nc = _build_program(row_segs, col_segs, plan)
    in_maps = _make_in_maps(input, row_segs, plan)
    return nc, in_maps, plan


def _make_in_maps(input, row_segs, plan):
    Mh = plan["Mh"]
    prT = np.zeros((H, Mh * P), dtype=np.float16)
    for m in range(Mh):
        chunk = plan["seg_perm"][m * P:(m + 1) * P]
        for j, s in enumerate(chunk):
            a, b = row_segs[s]
            prT[a:b, m * P + j] = np.float16(1.0 / (b - a))

    # device x layout: class blocks in slot order; within a class block
    # each segment is [c, l] (channel-major) so the device reduce over l
    # is contiguous.  wperm[lw0:lw0+n*L] gives the class's orig w cols.
    xp16 = np.asarray(input)[0].astype(np.float16)  # [H, W, C]
    parts = []  # per class: [H, n, C, L]
    for (L, n, lw0, slot0) in plan["col_runs"]:
        cols = plan["wperm"][lw0:lw0 + n * L]
        blk = xp16[:, cols, :].reshape(H, n, L, C)
        parts.append(np.ascontiguousarray(blk.transpose(0, 1, 3, 2)))
    # row-expansion one-hot: rE[m*P + j, r] = 1 iff device row r belongs
    # to the j-th (class-sorted) segment of s-chunk m
    Mh = plan["Mh"]
    rE = np.zeros((Mh * P, H), dtype=np.float16)
    for m in range(Mh):
        for (r0, n, L, j0) in plan["d_runs"][m]:
            for jj in range(n):
                rE[m * P + j0 + jj, r0 + jj * L:r0 + (jj + 1) * L] = 1.0

    in_maps = []
    for k in range(NCORES):
        xc = np.concatenate(
            [p[:, :, k * CS:(k + 1) * CS, :].reshape(H, -1) for p in parts],
            axis=1,
        )
        in_maps.append(
            {"x": np.ascontiguousarray(xc), "prT": prT, "rE": rE}
        )
    return in_maps


# stash for test.py introspection
LAST_RESULT = {}
_EXEC_CACHE = {}


def _make_executable(nc):
    """Build a reusable sharded jit callable for this program.

    Mirrors bass2jax.run_bass_via_pjrt's multi-core branch but keeps the
    jitted function so repeated calls skip retrace/recompile (and so the
    test harness can time steady-state executions).
    """
    import jax
    import concourse.mybir as mybir
    from concourse import bass2jax
    from jax.sharding import Mesh, PartitionSpec
    from jax.experimental.shard_map import shard_map

    bass2jax.install_neuronx_cc_hook()

    partition_name = (
        nc.partition_id_tensor.name if nc.partition_id_tensor else None
    )
    in_names, out_names, out_shapes, out_dtypes = [], [], [], []
    for alloc in nc.m.functions[0].allocations:
        if not isinstance(alloc, mybir.MemoryLocationSet):
            continue
        name = alloc.memorylocations[0].name
        if alloc.kind == "ExternalInput":
            if name != partition_name:
                in_names.append(name)
        elif alloc.kind == "ExternalOutput":
            out_names.append(name)
            out_shapes.append(tuple(alloc.tensor_shape))
            out_dtypes.append(mybir.dt.np(alloc.dtype))
    out_avals = tuple(
        jax.core.ShapedArray(s, d) for s, d in zip(out_shapes, out_dtypes)
    )
    n_params = len(in_names)
    n_outs = len(out_names)
    all_names = in_names + out_names
    if partition_name is not None:
        all_names = all_names + [partition_name]

    def _body(*args):
        operands = list(args)
        if partition_name is not None:
            operands.append(bass2jax.partition_id_tensor())
        outs = bass2jax._bass_exec_p.bind(
            *operands,
            out_avals=out_avals,
            in_names=tuple(all_names),
            out_names=tuple(out_names),
            lowering_input_output_aliases=(),
            sim_require_finite=True,
            sim_require_nnan=True,
            nc=nc,
        )
        return tuple(outs)

    devices = jax.devices()[:NCORES]
    mesh = Mesh(np.asarray(devices), ("core",))
    donate = tuple(range(n_params, n_params + n_outs))
    sharded = jax.jit(
        shard_map(
            _body,
            mesh=mesh,
            in_specs=(PartitionSpec("core"),) * (n_params + n_outs),
            out_specs=(PartitionSpec("core"),) * n_outs,
            check_rep=False,
        ),
        donate_argnums=donate,
        keep_unused=True,
    )

    def run(in_maps):
        concat_in = [
            np.concatenate([m[name] for m in in_maps], axis=0)
            for name in in_names
        ]
        concat_zeros = [
            np.zeros((NCORES * s[0], *s[1:]), d)
            for s, d in zip(out_shapes, out_dtypes)
        ]
        out_arrs = sharded(*concat_in, *concat_zeros)
        return [
            {
                name: np.asarray(out_arrs[i]).reshape(
                    NCORES, *out_shapes[i]
                )[c]
                for i, name in enumerate(out_names)
            }
            for c in range(NCORES)
        ]

    return run


def _get_run(input, h_mask, v_mask):
    key = (np.asarray(h_mask).tobytes(), np.asarray(v_mask).tobytes())
    if key not in _EXEC_CACHE:
        nc, in_maps, plan = _prep_host(
            np.asarray(input), np.asarray(h_mask), np.asarray(v_mask)
        )
        LAST_RESULT["nc"] = nc
        _EXEC_CACHE[key] = (_make_executable(nc), plan)
    else:
        row_segs = _segments(h_mask)
        plan = _EXEC_CACHE[key][1]
        in_maps = _make_in_maps(np.asarray(input), row_segs, plan)
    return _EXEC_CACHE[key][0], in_maps


def kernel(input, h_mask, v_mask):
    run, in_maps = _get_run(input, h_mask, v_mask)
    results = run(in_maps)
    LAST_RESULT["results"] = results

    key = (np.asarray(h_mask).tobytes(), np.asarray(v_mask).tobytes())
    plan = _EXEC_CACHE[key][1]
    # un-permute rows (class-grouped) and cols (class-sorted), upcast
    orig2dev = np.empty(H, dtype=np.int64)
    orig2dev[plan["dev_rows"]] = np.arange(H)
    winv = np.empty(W, dtype=np.int64)
    winv[plan["wperm"]] = np.arange(W)

    out = np.empty((H, W, C), dtype=np.float32)
    for k in range(NCORES):
        yk = results[k]["y"].reshape(H, W, CS)
        out[:, :, k * CS:(k + 1) * CS] = yk[orig2dev][:, winv]
    return out[None]


# revision 25
# speedup vs baseline: 71530.0723x; 1.0576x over previous
"""GridPoolingLayer kernel for Trainium2 (8 NeuronCores, Bass/Tile).

Semantics: the 1D binary masks partition H/W into maximal runs of constant
value; the layer replaces every grid cell with its mean (keep_size=True).
The op is separable: col-segment-mean along W, then row-segment-mean along
H, then broadcast back over each cell.

Device strategy per core (channels sharded 8 ways, 32 ch/core), fp16:

  B) col pooling   cp[k] = segment-sum_w(x chunk k)   -- DVE tensor_reduce
     along the free axis.  W is pre-permuted host-side so col segments of
     equal length are adjacent -> one reduce instruction per length class
     per load block.
  A) row pooling   pooled = P_r^T @ cp                -- PE matmul
     (contraction over H on partitions), P_r one-hot/len fp16 matrix
     precomputed host-side; row segments are permuted within each
     128-segment chunk so equal lengths are adjacent.
  C) col expand    rowtile[:, w] = pooled[:, seg(w)] / len_w -- DVE
     tensor_scalar_mul reading PSUM directly with a step-0 broadcast AP,
     one instruction per (length class x PSUM tile) piece.
  D) row expand    y rows = broadcast of pooled rows  -- DMA straight
     from SBUF with a step-0 source AP, ONE DMA per (s-chunk x row length
     class): output rows are written in class-grouped order.

The host un-permutes both axes (pure gathers) while unsharding and
upcasts fp16 -> fp32.  fp16 keeps HBM traffic at 16 MB in + 16 MB out
per core (vs 64 MB for fp32) and runs the PE at full 16-bit rate; the
2e-2 harness tolerance leaves ~40x margin over fp16 rounding noise.
"""

import math
import numpy as np

H, W, C = 512, 512, 256
NCORES = 8
CS = C // NCORES  # 32 channels per core
P = 128
FW = W * CS       # full row free size in elements (16384)
PSW = 512         # psum tile width (fp32 elems, one bank)
SLOTS_PER_TILE = PSW // CS  # 16 col segments per psum tile

TB = 128          # x load block target width (w units; 128 -> 1MB DMAs)
XIN_BUFS = 3
CHUNK2 = 1024     # psum2 tile width (fp32 elems, two banks)


def _segments(mask):
    m = np.asarray(mask).ravel()
    change = np.nonzero(m[1:] != m[:-1])[0] + 1
    bounds = np.concatenate([[0], change, [len(m)]]).astype(np.int64)
    return [(int(bounds[i]), int(bounds[i + 1])) for i in range(len(bounds) - 1)]


def _plan(row_segs, col_segs):
    """Host-side geometry planning shared by program build + data prep."""
    from collections import defaultdict

    S_h, S_w = len(row_segs), len(col_segs)
    Mh = math.ceil(S_h / P)
    Kh = math.ceil(H / P)

    # ---- column side: class-sorted device order -------------------------
    by_len = defaultdict(list)
    for t, (u, v) in enumerate(col_segs):
        by_len[v - u].append(t)
    perm_cols = [t for L in sorted(by_len) for t in by_len[L]]

    wperm = np.empty(W, dtype=np.int64)   # dev w unit -> orig w
    wstart = np.empty(S_w + 1, dtype=np.int64)  # dev slot -> dev w unit
    off = 0
    for sl, t in enumerate(perm_cols):
        u, v = col_segs[t]
        wstart[sl] = off
        wperm[off:off + (v - u)] = np.arange(u, v)
        off += v - u
    wstart[S_w] = off
    assert off == W

    # class runs in device slot order: (L, n, lw0, slot0)
    col_runs = []
    sl = 0
    for L in sorted(by_len):
        n = len(by_len[L])
        col_runs.append((L, n, int(wstart[sl]), sl))
        sl += n

    # ---- x load blocks over the L>=2 classes: whole classes, merged -----
    # until >= TB w units.  (The L==1 class is DMAed straight into the
    # col-pooled tile.)  Each class block is stored l-major [L, n, CS] so
    # the segment sum is L-1 fully contiguous tensor_tensor adds.
    load_blocks = []
    cur = {"w0": None, "wb": 0, "runs": []}
    for (L, n, lw0, slot0) in col_runs:
        if L == 1:
            continue
        if cur["w0"] is None:
            cur["w0"] = lw0
        cur["runs"].append((L, n, lw0 - cur["w0"], slot0))
        cur["wb"] += n * L
        if cur["wb"] >= TB:
            load_blocks.append(cur)
            cur = {"w0": None, "wb": 0, "runs": []}
    if cur["wb"]:
        load_blocks.append(cur)

    # ---- expansion pieces: class runs split at psum2 chunk boundaries ---
    # chunk = CHUNK2 fp32 elems (CHUNK2//CS slots); piece: (L, n, slot0, lw0)
    slots_per_chunk = CHUNK2 // CS
    exp_pieces = defaultdict(list)
    for (L, n, lw0, slot0) in col_runs:
        i = 0
        while i < n:
            ch = (slot0 + i) // slots_per_chunk
            room = (ch + 1) * slots_per_chunk - (slot0 + i)
            take = min(room, n - i)
            exp_pieces[ch].append(
                (L, take, slot0 + i, int(wstart[slot0 + i]))
            )
            i += take

    n_tiles = math.ceil(S_w * CS / PSW)
    n_chunks = math.ceil(S_w * CS / CHUNK2)

    # ---- row side: class-sorted order within each s-chunk ---------------
    # dev_rows[r] = orig h; d_runs[m] = [(r0, n, L, j0)]
    dev_rows = np.empty(H, dtype=np.int64)
    d_runs = [[] for _ in range(Mh)]
    seg_perm = []  # global seg order: for m, class-sorted within chunk
    r0 = 0
    for m in range(Mh):
        chunk = list(range(m * P, min(S_h, (m + 1) * P)))
        chunk.sort(key=lambda s: (row_segs[s][1] - row_segs[s][0], s))
        j = 0
        while j < len(chunk):
            L = row_segs[chunk[j]][1] - row_segs[chunk[j]][0]
            n = 0
            while j + n < len(chunk) and (
                row_segs[chunk[j + n]][1] - row_segs[chunk[j + n]][0] == L
            ):
                n += 1
            d_runs[m].append((r0, n, L, j))
            for jj in range(n):
                a, b = row_segs[chunk[j + jj]]
                dev_rows[r0:r0 + L] = np.arange(a, b)
                r0 += L
            j += n
        seg_perm.extend(chunk)
    assert r0 == H

    # ---- row chunk overlap: which h-chunks feed each s-chunk ------------
    overlap = []
    for m in range(Mh):
        s_lo, s_hi = m * P, min(S_h, (m + 1) * P)
        h_lo = min(row_segs[s][0] for s in range(s_lo, s_hi))
        h_hi = max(row_segs[s][1] for s in range(s_lo, s_hi))
        overlap.append(
            [k for k in range(Kh) if k * P < h_hi and (k + 1) * P > h_lo]
        )

    # ---- output windows: <=128 consecutive device rows, one s-chunk -----
    windows = []
    for m in range(Mh):
        R0 = d_runs[m][0][0]
        R1 = d_runs[m][-1][0] + d_runs[m][-1][1] * d_runs[m][-1][2]
        a = R0
        while a < R1:
            windows.append((m, a, min(P, R1 - a)))
            a += P

    return dict(
        S_h=S_h, S_w=S_w, Mh=Mh, Kh=Kh,
        wperm=wperm, col_runs=col_runs, load_blocks=load_blocks,
        exp_pieces=exp_pieces, n_tiles=n_tiles, n_chunks=n_chunks,
        dev_rows=dev_rows, d_runs=d_runs, seg_perm=seg_perm,
        overlap=overlap, windows=windows,
    )


def _build_program(row_segs, col_segs, plan):
    import concourse.bass as bass
    import concourse.mybir as mybir
    import concourse.tile as tile
    from concourse import bacc

    fp16 = mybir.dt.float16
    fp32 = mybir.dt.float32
    ADD = mybir.AluOpType.add
    AXX = mybir.AxisListType.X

    Mh, Kh = plan["Mh"], plan["Kh"]
    S_w = plan["S_w"]
    CPW = S_w * CS  # col-pooled row free size

    COPY = mybir.ActivationFunctionType.Copy

    nc = bacc.Bacc()
    x = nc.dram_tensor("x", [H, FW], fp16, kind="ExternalInput")
    prT = nc.dram_tensor("prT", [H, Mh * P], fp16, kind="ExternalInput")
    rE = nc.dram_tensor("rE", [Mh * P, H], fp16, kind="ExternalInput")
    y = nc.dram_tensor("y", [H, FW], fp16, kind="ExternalOutput")

    with tile.TileContext(nc) as tc:
        with (
            tc.tile_pool(name="consts", bufs=1) as consts,
            tc.tile_pool(name="xin", bufs=XIN_BUFS) as xin,
            tc.tile_pool(name="cp", bufs=1) as cppool,
            tc.tile_pool(name="pld", bufs=2) as pldpool,
            tc.tile_pool(name="ot", bufs=2) as otpool,
            tc.tile_pool(name="ps", bufs=2, space="PSUM") as pspool,
            tc.tile_pool(name="ps2", bufs=2, space="PSUM") as ps2pool,
            tc.tile_pool(name="warm", bufs=1, space="PSUM") as warmpool,
        ):
            # stationary matrices: row-pool prT per h-chunk, row-expand
            # one-hot rE per s-chunk
            prT_sb = []
            for k in range(Kh):
                t = consts.tile([P, Mh * P], fp16, name=f"prT{k}")
                nc.sync.dma_start(t[:], prT[k * P:(k + 1) * P, :])
                prT_sb.append(t)
            rE_sb = []
            for m in range(Mh):
                t = consts.tile([P, H], fp16, name=f"rE{m}")
                nc.sync.dma_start(t[:], rE[m * P:(m + 1) * P, :])
                rE_sb.append(t)

            # PE pre-touch of every stationary tile: later matmuls then
            # reach the operand without a DMA wait (keeps the LDWEIGHTS
            # sync-wait count within the ISA limit).
            ps_warm = warmpool.tile([1, PSW], fp32, name="ps_warm")
            for t in prT_sb + rE_sb:
                nc.tensor.matmul(
                    ps_warm[:1, :1],
                    t[:, :1],
                    t[:, :1],
                    start=True,
                    stop=True,
                )

            # ------------- phase B: load + col segment-sum ---------------
            # x element layout per class block: [j, c, l] (channel-major
            # inside each segment) so the reduce over l reads contiguous
            # fp16; the L==1 class is plain [j, c] and goes straight into
            # cp via DMA.  All loads ride the Activation HWDGE ring so
            # output stores keep the SP ring to themselves.
            cp = [
                cppool.tile([P, CPW], fp16, name=f"cp{k}") for k in range(Kh)
            ]
            one_run = next(
                ((L, n, lw0, slot0) for (L, n, lw0, slot0)
                 in plan["col_runs"] if L == 1), None
            )
            add_rr = 0
            for k in range(Kh):
                if one_run is not None:
                    L, n, lw0, slot0 = one_run
                    nc.scalar.dma_start(
                        cp[k][:, slot0 * CS:(slot0 + n) * CS],
                        x[k * P:(k + 1) * P, lw0 * CS:(lw0 + n) * CS],
                    )
                for bi, blk in enumerate(plan["load_blocks"]):
                    wb = blk["wb"]
                    xt = xin.tile([P, wb * CS], fp16, tag="xt",
                                  name=f"xt{k}_{bi}")
                    nc.scalar.dma_start(
                        xt[:],
                        x[k * P:(k + 1) * P,
                          blk["w0"] * CS:(blk["w0"] + wb) * CS],
                    )
                    # class block is [L, n, CS] l-major: the segment sum
                    # is a chain of L-1 contiguous adds, alternating
                    # between Vector and GpSimd (both SBUF-only here)
                    for (L, n, lw0, slot0) in blk["runs"]:
                        dst = cp[k][:, slot0 * CS:(slot0 + n) * CS]
                        sl0 = lw0 * CS
                        w = n * CS
                        eng = nc.vector if add_rr % 2 == 0 else nc.gpsimd
                        add_rr += 1
                        with nc.allow_low_precision(
                            reason="fp16 col sums; 2e-2 tolerance"
                        ):
                            for l in range(1, L):
                                in0 = (xt[:, sl0:sl0 + w] if l == 1
                                       else dst)
                                in1 = xt[:, sl0 + l * w:sl0 + (l + 1) * w]
                                eng.tensor_tensor(dst, in0, in1, op=ADD)

            # --------- phases A+D+C per s-chunk --------------------------
            # A: pool rows into psum1, copy to pooled (fp16, narrow).
            # D: row-expand the NARROW pooled tensor via one-hot PE matmul
            #    into 128-row output windows (PE reads pooled across all
            #    partitions in parallel -- a DMA broadcast source would be
            #    pinned to single SBUF ports, and expanding before the col
            #    expand halves the PE work and keeps its stream dense).
            # C: col-expand psum2 -> ot while downcasting, spread across
            #    Vector/GpSimd/Scalar.  One clean 2D store per window.
            exp_rr = 0
            for m in range(Mh):
                pld = pldpool.tile([P, CPW], fp16, tag="pld",
                                   name=f"pld{m}")
                ks = plan["overlap"][m]
                for t_idx in range(plan["n_tiles"]):
                    n0 = t_idx * PSW
                    nw = min(PSW, CPW - n0)
                    ps = pspool.tile([P, PSW], fp32, tag="ps",
                                     name=f"ps{m}_{t_idx}")
                    for i, k in enumerate(ks):
                        nc.tensor.matmul(
                            ps[:, :nw],
                            prT_sb[k][:, m * P:(m + 1) * P],
                            cp[k][:, n0:n0 + nw],
                            start=(i == 0),
                            stop=(i == len(ks) - 1),
                        )
                    nc.scalar.activation(
                        pld[:, n0:n0 + nw], ps[:, :nw], COPY
                    )
                for (wm, a, wlen) in plan["windows"]:
                    if wm != m:
                        continue
                    ot = otpool.tile([P, FW], fp16, tag="ot",
                                     name=f"ot{m}_{a}")
                    for ci in range(plan["n_chunks"]):
                        c0 = ci * CHUNK2
                        cw = min(CHUNK2, CPW - c0)
                        ps2 = ps2pool.tile([P, CHUNK2], fp32, tag="ps2",
                                           name=f"ps2_{m}_{a}_{ci}")
                        for h0 in range(0, cw, PSW):
                            hw = min(PSW, cw - h0)
                            nc.tensor.matmul(
                                ps2[:wlen, h0:h0 + hw],
                                rE_sb[m][:, a:a + wlen],
                                pld[:, c0 + h0:c0 + h0 + hw],
                                start=True,
                                stop=True,
                            )
                        for (L, n, slot0, lw0) in plan["exp_pieces"][ci]:
                            src = ps2[:wlen,
                                      slot0 * CS - c0:(slot0 + n) * CS - c0]
                            src = src.rearrange("p (j c) -> p j c",
                                                j=n, c=CS)
                            src = src.unsqueeze(2)
                            src = src.broadcast_to([wlen, n, L, CS])
                            dst = ot[:wlen, lw0 * CS:(lw0 + n * L) * CS]
                            dst = dst.rearrange("p (j l c) -> p j l c",
                                                j=n, l=L, c=CS)
                            if exp_rr % 2 == 0:
                                nc.vector.tensor_scalar_mul(dst, src,
                                                            1.0 / L)
                            else:
                                nc.scalar.activation(dst, src, COPY,
                                                     scale=1.0 / L)
                            exp_rr += 1
                    half = FW // 2
                    nc.sync.dma_start(y[a:a + wlen, :half],
                                      ot[:wlen, :half])
                    nc.sync.dma_start(y[a:a + wlen, half:],
                                      ot[:wlen, half:])

    nc.compile()
    nc.finalize()
    return nc


def _prep_host(input, h_mask, v_mask):
    """Returns (nc, in_maps, plan) ready for execution."""
    row_segs = _segments(h_mask)
    col_segs = _segments(v_mask)
    plan = _plan(row_segs, col_segs)
    nc = _build_program(row_segs, col_segs, plan)
    in_maps = _make_in_maps(input, row_segs, plan)
    return nc, in_maps, plan


def _make_in_maps(input, row_segs, plan):
    Mh = plan["Mh"]
    prT = np.zeros((H, Mh * P), dtype=np.float16)
    for m in range(Mh):
        chunk = plan["seg_perm"][m * P:(m + 1) * P]
        for j, s in enumerate(chunk):
            a, b = row_segs[s]
            prT[a:b, m * P + j] = np.float16(1.0 / (b - a))

    # device x layout: class blocks in slot order; each class block is
    # [L, n, C] l-major so the device segment sum is L-1 contiguous adds.
    # wperm[lw0:lw0+n*L] gives the class's orig w cols.
    xp16 = np.asarray(input)[0].astype(np.float16)  # [H, W, C]
    parts = []  # per class: [H, L, n, C]
    for (L, n, lw0, slot0) in plan["col_runs"]:
        cols = plan["wperm"][lw0:lw0 + n * L]
        blk = xp16[:, cols, :].reshape(H, n, L, C)
        parts.append(np.ascontiguousarray(blk.transpose(0, 2, 1, 3)))
    # row-expansion one-hot: rE[m*P + j, r] = 1 iff device row r belongs
    # to the j-th (class-sorted) segment of s-chunk m
    Mh = plan["Mh"]
    rE = np.zeros((Mh * P, H), dtype=np.float16)
    for m in range(Mh):
        for (r0, n, L, j0) in plan["d_runs"][m]:
            for jj in range(n):
                rE[m * P + j0 + jj, r0 + jj * L:r0 + (jj + 1) * L] = 1.0

    in_maps = []
    for k in range(NCORES):
        xc = np.concatenate(
            [p[:, :, :, k * CS:(k + 1) * CS].reshape(H, -1) for p in parts],
            axis=1,
        )
        in_maps.append(
            {"x": np.ascontiguousarray(xc), "prT": prT, "rE": rE}
        )
    return in_maps


# stash for test.py introspection
LAST_RESULT = {}
_EXEC_CACHE = {}


def _make_executable(nc):
    """Build a reusable sharded jit callable for this program.

    Mirrors bass2jax.run_bass_via_pjrt's multi-core branch but keeps the
    jitted function so repeated calls skip retrace/recompile (and so the
    test harness can time steady-state executions).
    """
    import jax
    import concourse.mybir as mybir
    from concourse import bass2jax
    from jax.sharding import Mesh, PartitionSpec
    from jax.experimental.shard_map import shard_map

    bass2jax.install_neuronx_cc_hook()

    partition_name = (
        nc.partition_id_tensor.name if nc.partition_id_tensor else None
    )
    in_names, out_names, out_shapes, out_dtypes = [], [], [], []
    for alloc in nc.m.functions[0].allocations:
        if not isinstance(alloc, mybir.MemoryLocationSet):
            continue
        name = alloc.memorylocations[0].name
        if alloc.kind == "ExternalInput":
            if name != partition_name:
                in_names.append(name)
        elif alloc.kind == "ExternalOutput":
            out_names.append(name)
            out_shapes.append(tuple(alloc.tensor_shape))
            out_dtypes.append(mybir.dt.np(alloc.dtype))
    out_avals = tuple(
        jax.core.ShapedArray(s, d) for s, d in zip(out_shapes, out_dtypes)
    )
    n_params = len(in_names)
    n_outs = len(out_names)
    all_names = in_names + out_names
    if partition_name is not None:
        all_names = all_names + [partition_name]

    def _body(*args):
        operands = list(args)
        if partition_name is not None:
            operands.append(bass2jax.partition_id_tensor())
        outs = bass2jax._bass_exec_p.bind(
            *operands,
            out_avals=out_avals,
            in_names=tuple(all_names),
            out_names=tuple(out_names),
            lowering_input_output_aliases=(),
            sim_require_finite=True,
            sim_require_nnan=True,
            nc=nc,
        )
        return tuple(outs)

    devices = jax.devices()[:NCORES]
    mesh = Mesh(np.asarray(devices), ("core",))
    donate = tuple(range(n_params, n_params + n_outs))
    sharded = jax.jit(
        shard_map(
            _body,
            mesh=mesh,
            in_specs=(PartitionSpec("core"),) * (n_params + n_outs),
            out_specs=(PartitionSpec("core"),) * n_outs,
            check_rep=False,
        ),
        donate_argnums=donate,
        keep_unused=True,
    )

    def run(in_maps):
        concat_in = [
            np.concatenate([m[name] for m in in_maps], axis=0)
            for name in in_names
        ]
        concat_zeros = [
            np.zeros((NCORES * s[0], *s[1:]), d)
            for s, d in zip(out_shapes, out_dtypes)
        ]
        out_arrs = sharded(*concat_in, *concat_zeros)
        return [
            {
                name: np.asarray(out_arrs[i]).reshape(
                    NCORES, *out_shapes[i]
                )[c]
                for i, name in enumerate(out_names)
            }
            for c in range(NCORES)
        ]

    return run


def _get_run(input, h_mask, v_mask):
    key = (np.asarray(h_mask).tobytes(), np.asarray(v_mask).tobytes())
    if key not in _EXEC_CACHE:
        nc, in_maps, plan = _prep_host(
            np.asarray(input), np.asarray(h_mask), np.asarray(v_mask)
        )
        LAST_RESULT["nc"] = nc
        _EXEC_CACHE[key] = (_make_executable(nc), plan)
    else:
        row_segs = _segments(h_mask)
        plan = _EXEC_CACHE[key][1]
        in_maps = _make_in_maps(np.asarray(input), row_segs, plan)
    return _EXEC_CACHE[key][0], in_maps


def kernel(input, h_mask, v_mask):
    run, in_maps = _get_run(input, h_mask, v_mask)
    results = run(in_maps)
    LAST_RESULT["results"] = results

    key = (np.asarray(h_mask).tobytes(), np.asarray(v_mask).tobytes())
    plan = _EXEC_CACHE[key][1]
    # un-permute rows (class-grouped) and cols (class-sorted), upcast
    orig2dev = np.empty(H, dtype=np.int64)
    orig2dev[plan["dev_rows"]] = np.arange(H)
    winv = np.empty(W, dtype=np.int64)
    winv[plan["wperm"]] = np.arange(W)

    out = np.empty((H, W, C), dtype=np.float32)
    for k in range(NCORES):
        yk = results[k]["y"].reshape(H, W, CS)
        out[:, :, k * CS:(k + 1) * CS] = yk[orig2dev][:, winv]
    return out[None]


# revision 30
# speedup vs baseline: 73712.6430x; 1.0305x over previous
"""GridPoolingLayer kernel for Trainium2 (8 NeuronCores, Bass/Tile).

Semantics: the 1D binary masks partition H/W into maximal runs of constant
value; the layer replaces every grid cell with its mean (keep_size=True).
The op is separable: col-segment-mean along W, then row-segment-mean along
H, then broadcast back over each cell.

Device strategy per core (channels sharded 8 ways, 32 ch/core), fp16:

  B) col pooling   cp[k] = segment-sum_w(x chunk k)   -- DVE tensor_reduce
     along the free axis.  W is pre-permuted host-side so col segments of
     equal length are adjacent -> one reduce instruction per length class
     per load block.
  A) row pooling   pooled = P_r^T @ cp                -- PE matmul
     (contraction over H on partitions), P_r one-hot/len fp16 matrix
     precomputed host-side; row segments are permuted within each
     128-segment chunk so equal lengths are adjacent.
  C) col expand    rowtile[:, w] = pooled[:, seg(w)] / len_w -- DVE
     tensor_scalar_mul reading PSUM directly with a step-0 broadcast AP,
     one instruction per (length class x PSUM tile) piece.
  D) row expand    y rows = broadcast of pooled rows  -- DMA straight
     from SBUF with a step-0 source AP, ONE DMA per (s-chunk x row length
     class): output rows are written in class-grouped order.

The host un-permutes both axes (pure gathers) while unsharding and
upcasts fp16 -> fp32.  fp16 keeps HBM traffic at 16 MB in + 16 MB out
per core (vs 64 MB for fp32) and runs the PE at full 16-bit rate; the
2e-2 harness tolerance leaves ~40x margin over fp16 rounding noise.
"""

import math
import numpy as np

H, W, C = 512, 512, 256
NCORES = 8
CS = C // NCORES  # 32 channels per core
P = 128
FW = W * CS       # full row free size in elements (16384)
PSW = 512         # psum tile width (fp32 elems, one bank)
SLOTS_PER_TILE = PSW // CS  # 16 col segments per psum tile

TB = 128          # x load block target width (w units; 128 -> 1MB DMAs)
XIN_BUFS = 3
CHUNK2 = 1024     # psum2 tile width (fp32 elems, two banks)


def _segments(mask):
    m = np.asarray(mask).ravel()
    change = np.nonzero(m[1:] != m[:-1])[0] + 1
    bounds = np.concatenate([[0], change, [len(m)]]).astype(np.int64)
    return [(int(bounds[i]), int(bounds[i + 1])) for i in range(len(bounds) - 1)]


def _plan(row_segs, col_segs):
    """Host-side geometry planning shared by program build + data prep."""
    from collections import defaultdict

    S_h, S_w = len(row_segs), len(col_segs)
    Mh = math.ceil(S_h / P)
    Kh = math.ceil(H / P)

    # ---- column side: class-sorted device order -------------------------
    by_len = defaultdict(list)
    for t, (u, v) in enumerate(col_segs):
        by_len[v - u].append(t)
    perm_cols = [t for L in sorted(by_len) for t in by_len[L]]

    wperm = np.empty(W, dtype=np.int64)   # dev w unit -> orig w
    wstart = np.empty(S_w + 1, dtype=np.int64)  # dev slot -> dev w unit
    off = 0
    for sl, t in enumerate(perm_cols):
        u, v = col_segs[t]
        wstart[sl] = off
        wperm[off:off + (v - u)] = np.arange(u, v)
        off += v - u
    wstart[S_w] = off
    assert off == W

    # class runs in device slot order: (L, n, lw0, slot0)
    col_runs = []
    sl = 0
    for L in sorted(by_len):
        n = len(by_len[L])
        col_runs.append((L, n, int(wstart[sl]), sl))
        sl += n

    # ---- x load blocks over the L>=2 classes: whole classes, merged -----
    # until >= TB w units.  (The L==1 class is DMAed straight into the
    # col-pooled tile.)  Each class block is stored l-major [L, n, CS] so
    # the segment sum is L-1 fully contiguous tensor_tensor adds.
    load_blocks = []
    cur = {"w0": None, "wb": 0, "runs": []}
    for (L, n, lw0, slot0) in col_runs:
        if L == 1:
            continue
        if cur["w0"] is None:
            cur["w0"] = lw0
        cur["runs"].append((L, n, lw0 - cur["w0"], slot0))
        cur["wb"] += n * L
        if cur["wb"] >= TB:
            load_blocks.append(cur)
            cur = {"w0": None, "wb": 0, "runs": []}
    if cur["wb"]:
        load_blocks.append(cur)

    # ---- expansion pieces: class runs split at psum2 chunk boundaries ---
    # chunk = CHUNK2 fp32 elems (CHUNK2//CS slots); piece: (L, n, slot0, lw0)
    slots_per_chunk = CHUNK2 // CS
    exp_pieces = defaultdict(list)
    for (L, n, lw0, slot0) in col_runs:
        i = 0
        while i < n:
            ch = (slot0 + i) // slots_per_chunk
            room = (ch + 1) * slots_per_chunk - (slot0 + i)
            take = min(room, n - i)
            exp_pieces[ch].append(
                (L, take, slot0 + i, int(wstart[slot0 + i]))
            )
            i += take

    n_tiles = math.ceil(S_w * CS / PSW)
    n_chunks = math.ceil(S_w * CS / CHUNK2)

    # ---- row side: class-sorted order within each s-chunk ---------------
    # dev_rows[r] = orig h; d_runs[m] = [(r0, n, L, j0)]
    dev_rows = np.empty(H, dtype=np.int64)
    d_runs = [[] for _ in range(Mh)]
    seg_perm = []  # global seg order: for m, class-sorted within chunk
    r0 = 0
    for m in range(Mh):
        chunk = list(range(m * P, min(S_h, (m + 1) * P)))
        chunk.sort(key=lambda s: (row_segs[s][1] - row_segs[s][0], s))
        j = 0
        while j < len(chunk):
            L = row_segs[chunk[j]][1] - row_segs[chunk[j]][0]
            n = 0
            while j + n < len(chunk) and (
                row_segs[chunk[j + n]][1] - row_segs[chunk[j + n]][0] == L
            ):
                n += 1
            d_runs[m].append((r0, n, L, j))
            for jj in range(n):
                a, b = row_segs[chunk[j + jj]]
                dev_rows[r0:r0 + L] = np.arange(a, b)
                r0 += L
            j += n
        seg_perm.extend(chunk)
    assert r0 == H

    # ---- row chunk overlap: which h-chunks feed each s-chunk ------------
    overlap = []
    for m in range(Mh):
        s_lo, s_hi = m * P, min(S_h, (m + 1) * P)
        h_lo = min(row_segs[s][0] for s in range(s_lo, s_hi))
        h_hi = max(row_segs[s][1] for s in range(s_lo, s_hi))
        overlap.append(
            [k for k in range(Kh) if k * P < h_hi and (k + 1) * P > h_lo]
        )

    # ---- output windows: <=128 consecutive device rows, one s-chunk -----
    windows = []
    for m in range(Mh):
        R0 = d_runs[m][0][0]
        R1 = d_runs[m][-1][0] + d_runs[m][-1][1] * d_runs[m][-1][2]
        a = R0
        while a < R1:
            windows.append((m, a, min(P, R1 - a)))
            a += P

    return dict(
        S_h=S_h, S_w=S_w, Mh=Mh, Kh=Kh,
        wperm=wperm, col_runs=col_runs, load_blocks=load_blocks,
        exp_pieces=exp_pieces, n_tiles=n_tiles, n_chunks=n_chunks,
        dev_rows=dev_rows, d_runs=d_runs, seg_perm=seg_perm,
        overlap=overlap, windows=windows,
    )


def _build_program(row_segs, col_segs, plan):
    import concourse.bass as bass
    import concourse.mybir as mybir
    import concourse.tile as tile
    from concourse import bacc

    fp16 = mybir.dt.float16
    fp32 = mybir.dt.float32
    ADD = mybir.AluOpType.add
    AXX = mybir.AxisListType.X

    Mh, Kh = plan["Mh"], plan["Kh"]
    S_w = plan["S_w"]
    CPW = S_w * CS  # col-pooled row free size

    COPY = mybir.ActivationFunctionType.Copy

    nc = bacc.Bacc()
    x = nc.dram_tensor("x", [H, FW], fp16, kind="ExternalInput")
    # fused pool+expand matrix: F[h, r] = 1/L_h iff orig row h and device
    # output row r share a row segment (F = P_r^T pool followed by one-hot
    # row expansion, collapsed into a single PE stage)
    fM = nc.dram_tensor("fM", [H, H], fp16, kind="ExternalInput")
    y = nc.dram_tensor("y", [H, FW], fp16, kind="ExternalOutput")

    with tile.TileContext(nc) as tc:
        with (
            tc.tile_pool(name="consts", bufs=1) as consts,
            tc.tile_pool(name="xin", bufs=XIN_BUFS) as xin,
            tc.tile_pool(name="cp", bufs=1) as cppool,
            tc.tile_pool(name="ot", bufs=2) as otpool,
            tc.tile_pool(name="ps2", bufs=3, space="PSUM") as ps2pool,
            tc.tile_pool(name="warm", bufs=1, space="PSUM") as warmpool,
        ):
            # stationary fused pool+expand matrix, one tile per h-chunk
            fM_sb = []
            for k in range(Kh):
                t = consts.tile([P, H], fp16, name=f"fM{k}")
                nc.sync.dma_start(t[:], fM[k * P:(k + 1) * P, :])
                fM_sb.append(t)

            # PE pre-touch of every stationary tile: later matmuls then
            # reach the operand without a DMA wait (keeps the LDWEIGHTS
            # sync-wait count within the ISA limit).
            ps_warm = warmpool.tile([1, PSW], fp32, name="ps_warm")
            for t in fM_sb:
                nc.tensor.matmul(
                    ps_warm[:1, :1],
                    t[:, :1],
                    t[:, :1],
                    start=True,
                    stop=True,
                )

            # ------------- phase B: load + col segment-sum ---------------
            # x element layout per class block: [j, c, l] (channel-major
            # inside each segment) so the reduce over l reads contiguous
            # fp16; the L==1 class is plain [j, c] and goes straight into
            # cp via DMA.  All loads ride the Activation HWDGE ring so
            # output stores keep the SP ring to themselves.
            cp = [
                cppool.tile([P, CPW], fp16, name=f"cp{k}") for k in range(Kh)
            ]
            one_run = next(
                ((L, n, lw0, slot0) for (L, n, lw0, slot0)
                 in plan["col_runs"] if L == 1), None
            )
            add_rr = 0
            for k in range(Kh):
                if one_run is not None:
                    L, n, lw0, slot0 = one_run
                    nc.scalar.dma_start(
                        cp[k][:, slot0 * CS:(slot0 + n) * CS],
                        x[k * P:(k + 1) * P, lw0 * CS:(lw0 + n) * CS],
                    )
                for bi, blk in enumerate(plan["load_blocks"]):
                    wb = blk["wb"]
                    xt = xin.tile([P, wb * CS], fp16, tag="xt",
                                  name=f"xt{k}_{bi}")
                    nc.scalar.dma_start(
                        xt[:],
                        x[k * P:(k + 1) * P,
                          blk["w0"] * CS:(blk["w0"] + wb) * CS],
                    )
                    # class block is [L, n, CS] l-major: the segment sum
                    # is a chain of L-1 contiguous adds, alternating
                    # between Vector and GpSimd (both SBUF-only here)
                    for (L, n, lw0, slot0) in blk["runs"]:
                        dst = cp[k][:, slot0 * CS:(slot0 + n) * CS]
                        sl0 = lw0 * CS
                        w = n * CS
                        eng = nc.vector if add_rr % 2 == 0 else nc.gpsimd
                        add_rr += 1
                        with nc.allow_low_precision(
                            reason="fp16 col sums; 2e-2 tolerance"
                        ):
                            for l in range(1, L):
                                in0 = (xt[:, sl0:sl0 + w] if l == 1
                                       else dst)
                                in1 = xt[:, sl0 + l * w:sl0 + (l + 1) * w]
                                eng.tensor_tensor(dst, in0, in1, op=ADD)

            # --------- fused pool+expand per output window ---------------
            # One PE stage: psum2[r, wseg] = sum_h F[h, r] * cp[h, wseg]
            # accumulated over h-chunks (F carries 1/L_h and the one-hot
            # row scatter).  Then col-expand psum2 -> ot while downcasting
            # (Vector/Scalar split; a DMA broadcast source would be pinned
            # to single SBUF ports).  Two clean 2D stores per window.
            exp_rr = 0
            for (m, a, wlen) in plan["windows"]:
                ks = plan["overlap"][m]
                ot = otpool.tile([P, FW], fp16, tag="ot", name=f"ot{a}")
                for ci in range(plan["n_chunks"]):
                    c0 = ci * CHUNK2
                    cw = min(CHUNK2, CPW - c0)
                    ps2 = ps2pool.tile([P, CHUNK2], fp32, tag="ps2",
                                       name=f"ps2_{a}_{ci}")
                    for h0 in range(0, cw, PSW):
                        hw = min(PSW, cw - h0)
                        for i, k in enumerate(ks):
                            nc.tensor.matmul(
                                ps2[:wlen, h0:h0 + hw],
                                fM_sb[k][:, a:a + wlen],
                                cp[k][:, c0 + h0:c0 + h0 + hw],
                                start=(i == 0),
                                stop=(i == len(ks) - 1),
                            )
                    for (L, n, slot0, lw0) in plan["exp_pieces"][ci]:
                        src = ps2[:wlen,
                                  slot0 * CS - c0:(slot0 + n) * CS - c0]
                        src = src.rearrange("p (j c) -> p j c",
                                            j=n, c=CS)
                        src = src.unsqueeze(2)
                        src = src.broadcast_to([wlen, n, L, CS])
                        dst = ot[:wlen, lw0 * CS:(lw0 + n * L) * CS]
                        dst = dst.rearrange("p (j l c) -> p j l c",
                                            j=n, l=L, c=CS)
                        if exp_rr % 2 == 0:
                            nc.vector.tensor_scalar_mul(dst, src,
                                                        1.0 / L)
                        else:
                            nc.scalar.activation(dst, src, COPY,
                                                 scale=1.0 / L)
                        exp_rr += 1
                half = FW // 2
                nc.sync.dma_start(y[a:a + wlen, :half],
                                  ot[:wlen, :half])
                nc.sync.dma_start(y[a:a + wlen, half:],
                                  ot[:wlen, half:])

    nc.compile()
    nc.finalize()
    return nc


def _prep_host(input, h_mask, v_mask):
    """Returns (nc, in_maps, plan) ready for execution."""
    row_segs = _segments(h_mask)
    col_segs = _segments(v_mask)
    plan = _plan(row_segs, col_segs)
    nc = _build_program(row_segs, col_segs, plan)
    in_maps = _make_in_maps(input, row_segs, plan)
    return nc, in_maps, plan


def _make_in_maps(input, row_segs, plan):
    # fused pool+expand matrix: F[h, r] = 1/L_h iff orig row h shares a
    # row segment with device output row r
    seg_of_h = np.empty(H, dtype=np.int64)
    seg_len = np.empty(H, dtype=np.int64)
    for s, (va, vb) in enumerate(row_segs):
        seg_of_h[va:vb] = s
        seg_len[va:vb] = vb - va
    seg_of_r = seg_of_h[plan["dev_rows"]]
    fM = np.where(
        seg_of_h[:, None] == seg_of_r[None, :],
        (1.0 / seg_len)[:, None],
        0.0,
    ).astype(np.float16)

    # device x layout: class blocks in slot order; each class block is
    # [L, n, C] l-major so the device segment sum is L-1 contiguous adds.
    # wperm[lw0:lw0+n*L] gives the class's orig w cols.
    xp16 = np.asarray(input)[0].astype(np.float16)  # [H, W, C]
    parts = []  # per class: [H, L, n, C]
    for (L, n, lw0, slot0) in plan["col_runs"]:
        cols = plan["wperm"][lw0:lw0 + n * L]
        blk = xp16[:, cols, :].reshape(H, n, L, C)
        parts.append(np.ascontiguousarray(blk.transpose(0, 2, 1, 3)))
    in_maps = []
    for k in range(NCORES):
        xc = np.concatenate(
            [p[:, :, :, k * CS:(k + 1) * CS].reshape(H, -1) for p in parts],
            axis=1,
        )
        in_maps.append({"x": np.ascontiguousarray(xc), "fM": fM})
    return in_maps


# stash for test.py introspection
LAST_RESULT = {}
_EXEC_CACHE = {}


def _make_executable(nc):
    """Build a reusable sharded jit callable for this program.

    Mirrors bass2jax.run_bass_via_pjrt's multi-core branch but keeps the
    jitted function so repeated calls skip retrace/recompile (and so the
    test harness can time steady-state executions).
    """
    import jax
    import concourse.mybir as mybir
    from concourse import bass2jax
    from jax.sharding import Mesh, PartitionSpec
    from jax.experimental.shard_map import shard_map

    bass2jax.install_neuronx_cc_hook()

    partition_name = (
        nc.partition_id_tensor.name if nc.partition_id_tensor else None
    )
    in_names, out_names, out_shapes, out_dtypes = [], [], [], []
    for alloc in nc.m.functions[0].allocations:
        if not isinstance(alloc, mybir.MemoryLocationSet):
            continue
        name = alloc.memorylocations[0].name
        if alloc.kind == "ExternalInput":
            if name != partition_name:
                in_names.append(name)
        elif alloc.kind == "ExternalOutput":
            out_names.append(name)
            out_shapes.append(tuple(alloc.tensor_shape))
            out_dtypes.append(mybir.dt.np(alloc.dtype))
    out_avals = tuple(
        jax.core.ShapedArray(s, d) for s, d in zip(out_shapes, out_dtypes)
    )
    n_params = len(in_names)
    n_outs = len(out_names)
    all_names = in_names + out_names
    if partition_name is not None:
        all_names = all_names + [partition_name]

    def _body(*args):
        operands = list(args)
        if partition_name is not None:
            operands.append(bass2jax.partition_id_tensor())
        outs = bass2jax._bass_exec_p.bind(
            *operands,
            out_avals=out_avals,
            in_names=tuple(all_names),
            out_names=tuple(out_names),
            lowering_input_output_aliases=(),
            sim_require_finite=True,
            sim_require_nnan=True,
            nc=nc,
        )
        return tuple(outs)

    devices = jax.devices()[:NCORES]
    mesh = Mesh(np.asarray(devices), ("core",))
    donate = tuple(range(n_params, n_params + n_outs))
    sharded = jax.jit(
        shard_map(
            _body,
            mesh=mesh,
            in_specs=(PartitionSpec("core"),) * (n_params + n_outs),
            out_specs=(PartitionSpec("core"),) * n_outs,
            check_rep=False,
        ),
        donate_argnums=donate,
        keep_unused=True,
    )

    def run(in_maps):
        concat_in = [
            np.concatenate([m[name] for m in in_maps], axis=0)
            for name in in_names
        ]
        concat_zeros = [
            np.zeros((NCORES * s[0], *s[1:]), d)
            for s, d in zip(out_shapes, out_dtypes)
        ]
        out_arrs = sharded(*concat_in, *concat_zeros)
        return [
            {
                name: np.asarray(out_arrs[i]).reshape(
                    NCORES, *out_shapes[i]
                )[c]
                for i, name in enumerate(out_names)
            }
            for c in range(NCORES)
        ]

    return run


def _get_run(input, h_mask, v_mask):
    key = (np.asarray(h_mask).tobytes(), np.asarray(v_mask).tobytes())
    if key not in _EXEC_CACHE:
        nc, in_maps, plan = _prep_host(
            np.asarray(input), np.asarray(h_mask), np.asarray(v_mask)
        )
        LAST_RESULT["nc"] = nc
        _EXEC_CACHE[key] = (_make_executable(nc), plan)
    else:
        row_segs = _segments(h_mask)
        plan = _EXEC_CACHE[key][1]
        in_maps = _make_in_maps(np.asarray(input), row_segs, plan)
    return _EXEC_CACHE[key][0], in_maps


def kernel(input, h_mask, v_mask):
    run, in_maps = _get_run(input, h_mask, v_mask)
    results = run(in_maps)
    LAST_RESULT["results"] = results

    key = (np.asarray(h_mask).tobytes(), np.asarray(v_mask).tobytes())
    plan = _EXEC_CACHE[key][1]
    # un-permute rows (class-grouped) and cols (class-sorted), upcast
    orig2dev = np.empty(H, dtype=np.int64)
    orig2dev[plan["dev_rows"]] = np.arange(H)
    winv = np.empty(W, dtype=np.int64)
    winv[plan["wperm"]] = np.arange(W)

    out = np.empty((H, W, C), dtype=np.float32)
    for k in range(NCORES):
        yk = results[k]["y"].reshape(H, W, CS)
        out[:, :, k * CS:(k + 1) * CS] = yk[orig2dev][:, winv]
    return out[None]
